# revision 18
# baseline (speedup 1.0000x reference)
"""Trainium2 Bass kernel for nn_Attention3D_fusion (cross-attention block).

Reference computation (B=16, N=1024, C=512, H=8, D=64):
    q = (x2 @ Wq.T) -> [B,H,N,D]  (queries from x2)
    k = (x  @ Wk.T) -> [B,H,N,D]
    v = (x  @ Wv.T) -> [B,H,N,D]
    attn = softmax(q @ k.T * D**-0.5)
    out  = (attn @ v) merged heads -> [B,N,C]
    y    = out @ Wp.T + bp
Sharding: batch data-parallel across 8 NeuronCores (2 batches/core), weights
replicated, no collectives.

Per-core kernel strategy (v3):
  - Inputs arrive host-side pre-transposed to [C, N], bf16, partition-major,
    and split into two contiguous 512-token halves [IH, P, CB, 512] so each
    half DMAs as one fully sequential 0.5MB read.
  - Engine budget per core (trace-derived): ACT does only exp, 128 tiles x
    1.11us = 142us; PE slots = attention 96us + projections 55us.  Both are
    at their rooflines; the kernel's job is overlap: wall ~= first-exp time
    + max(ACT chain, PE work) + drain.
  - DMA: two HWDGE rings (sync + scalar engines) at ~150GB/s each carry all
    critical loads, interleaved so q/k projections for head-pair 0 can start
    ~14us; wv + bias ride the slow gpsimd SWDGE ring, which forces the first
    attention iteration's PV matmuls to be deferred until after its exps
    (their results just accumulate later - the exp stream doesn't wait).
    b1's inputs follow on the rings with no gating (FIFO after b0's), and
    all 16 y-tile stores go out on the sync ring (engine otherwise idle) -
    the gpsimd SWDGE drain was 3us of tail in v2.
  - Scores are computed transposed with the two heads of a pair row-packed;
    softmax denominators come free as PV-output rows 0..63 via a 64-wide
    ones block in the v tiles; softmax skips max-subtraction (scores
    ~N(0,0.33), exp cannot overflow).
  - Fill pacing is demand-aware: b0's attention hides b0's remaining
    projections + all of b1's q/k/v prologue; b1's attention hides b0's
    output projection; only y(b1, second half) drains after the last exp.

Measured v2 (8 cores, NTFF): 211.5us.  v3 target ~185us.
"""

import os
import sys

import numpy as np

for _p in ("/opt/trn_rl_repo", "/root/.axon_site/_ro/trn_rl_repo"):
    if os.path.isdir(_p) and _p not in sys.path:
        sys.path.insert(0, _p)

import concourse.bass as bass
import concourse.tile as tile
from concourse import bacc, mybir
from concourse.bass_utils import run_bass_kernel_spmd

B, N, C = 16, 1024, 512
H, D = 8, 64
P = 128
NCORES = 8
B_LOC = B // NCORES  # batches per core
NB = N // P          # 8 token blocks
CB = C // P          # 4 channel blocks (also head-pairs: one block = 2 heads)
IH = N // 512        # 2 query/token halves of 512
SCALE = float(D) ** -0.5
F32 = mybir.dt.float32
BF16 = mybir.dt.bfloat16
EXP = mybir.ActivationFunctionType.Exp

_CACHE = {}


def _build_program():
    nc = bacc.Bacc("TRN2", target_bir_lowering=False, debug=False)

    # Inputs pre-transposed to [C, N] bf16 and arranged token-half-major
    # [IH, P, CB, 512]: each half is one contiguous 0.5MB block with 4KB
    # per-partition lines -> full-rate sequential DRAM reads, and the two
    # halves can ride different DGE rings concurrently.
    xts = nc.dram_tensor("xts", (B_LOC, IH, P, CB, 512), BF16, kind="ExternalInput").ap()
    x2ts = nc.dram_tensor("x2ts", (B_LOC, IH, P, CB, 512), BF16, kind="ExternalInput").ap()
    wqt = nc.dram_tensor("wqt", (P, CB, C), BF16, kind="ExternalInput").ap()
    wkt = nc.dram_tensor("wkt", (P, CB, C), BF16, kind="ExternalInput").ap()
    wvt = nc.dram_tensor("wvt", (P, CB, C), BF16, kind="ExternalInput").ap()
    wpt = nc.dram_tensor("wpt", (P, CB, C), BF16, kind="ExternalInput").ap()
    bp = nc.dram_tensor("bp", (C,), F32, kind="ExternalInput").ap()
    y = nc.dram_tensor("y", (B_LOC, N, C), F32, kind="ExternalOutput").ap()

    with tile.TileContext(nc) as tc:
        with (
            tc.tile_pool(name="consts", bufs=1) as consts,
            tc.tile_pool(name="big", bufs=2) as big,
            tc.tile_pool(name="ptp", bufs=9) as ptp,
            tc.tile_pool(name="ypool", bufs=3) as ypool,
            tc.tile_pool(name="rpool", bufs=4) as rpool,
            tc.tile_pool(name="mmout", bufs=2, space="PSUM") as mmout,
            tc.tile_pool(name="stp", bufs=2, space="PSUM") as stp,
            tc.tile_pool(name="avp", bufs=2, space="PSUM") as avp,
        ):
            # Pre-warm the ACT exp table (~2.7us ACT_TABLE_LOAD) before any
            # scores exist, so the first real exp doesn't pay it.
            warm = consts.tile([1, 16], F32, tag="warm", name="warm")
            nc.vector.memset(warm, 0.0)

            dummy = consts.tile([P, 640], BF16, tag="dummy", name="dummy")
            nc.vector.memset(dummy, 0.125)

            # Weight SBUF tiles.
            wsb = {
                name: consts.tile([P, CB, C], BF16, tag=f"w_{name}", name=f"w_{name}")
                for name in ("wq", "wk", "wv", "wp")
            }

            # Input tiles [P, IH, CB, 512] per batch, loaded as two
            # half-tensor DMAs each.
            state = {}

            def in_tile(b, which):
                st = state.setdefault(b, {})
                if which not in st:
                    st[which] = big.tile(
                        [P, IH, CB, 512], BF16, tag=which, name=f"{which}_b{b}"
                    )
                return st[which]

            def dma_half(b, which, h, eng, gate=None):
                """Load one token half.  `gate`: a produced 2-element
                region; a corner copy from it into the destination makes
                the DMA trigger wait - the SDMA engines round-robin across
                ALL in-flight transfers on a ring, so an ungated transfer
                steals bandwidth from the critical lead-in set."""
                src = x2ts if which == "x2T" else xts
                t = in_tile(b, which)
                if gate is not None:
                    nc.vector.tensor_copy(t[0:1, h, 0, 0:2], gate)
                eng.dma_start(out=t[:, h], in_=src[b, h])

            # --- DMA plan.  Triggers are the first user instructions on
            # each ring so descriptors hit the queues the moment the
            # preamble barrier clears.  Critical wave (ungated): the four
            # 0.5MB chunks the first q/k projections need, two per HWDGE
            # ring so all land ~14us.  Everything else is corner-gated
            # (sync ring only - a gated trigger on the scalar ring would
            # block the exp stream) or rides the slow gpsimd SWDGE.
            nc.sync.dma_start(out=wsb["wq"], in_=wqt)
            nc.scalar.dma_start(out=wsb["wk"], in_=wkt)
            dma_half(0, "x2T", 0, nc.sync)
            dma_half(0, "xT", 0, nc.scalar)
            nc.gpsimd.dma_start(out=wsb["wv"], in_=wvt)

            bias_bc = consts.tile([P, C], F32, tag="bias_bc", name="bias_bc")
            nc.gpsimd.dma_start(
                out=bias_bc,
                in_=bass.AP(tensor=bp.tensor, offset=bp.offset, ap=[[0, P], [1, C]]),
            )
            nc.gpsimd.dma_start(out=wsb["wp"], in_=wpt)

            # ACT exp-table warm (scalar engine, after its dma triggers).
            warm2 = consts.tile([1, 16], F32, tag="warm2", name="warm2")
            nc.scalar.activation(warm2, warm, EXP, scale=SCALE)

            # Persistent per-(batch, token-block) v tiles [P, H, ones|d].
            VT = {
                b: [
                    consts.tile(
                        [P, H, 2 * D], BF16, tag=f"VT{b}_{nb}", name=f"VT{b}_{nb}"
                    )
                    for nb in range(NB)
                ]
                for b in range(B_LOC)
            }

            def vt_memset(b, nb):
                nc.vector.memset(VT[b][nb][:, :, 0:D], 1.0)

            for nb in range(NB):
                vt_memset(0, nb)

            # Dummy-matmul bridge: keeps the PE HAM activity window filled
            # from preamble end (~3.6us) to the first real projection
            # (~14us at the cold 1.2GHz clock), so the clock flips to full
            # rate right as attention begins.
            dps = mmout.tile([P, 512], F32, tag="mm", name="dps")
            for i in range(20):
                nc.tensor.matmul(
                    dps, dummy[:, 0:P], dummy[:, P : P + 512],
                    start=(i == 0), stop=(i == 19),
                )

            def qk_one(b, wname, kb, ih, cp=None):
                """One q/k projection step: [P, 512] of transposed output."""
                if cp is None:
                    cp = nc.vector.tensor_copy
                st = state.setdefault(b, {})
                kind = "qT" if wname == "wq" else "kT"
                skey = "x2T" if wname == "wq" else "xT"
                dst = st.setdefault(kind, {})
                if kb not in dst:
                    dst[kb] = big.tile(
                        [P, N], BF16, tag=f"{kind}{kb}", name=f"{kind}{kb}_b{b}"
                    )

                def qk_step():
                    srcT = state[b][skey]
                    ps = mmout.tile(
                        [P, 512], F32, tag="mm", name=f"ps_{kind}_{b}_{kb}_{ih}"
                    )
                    for cb in range(CB):
                        nc.tensor.matmul(
                            ps,
                            wsb[wname][:, cb, kb * P : (kb + 1) * P],
                            srcT[:, ih, cb, :],
                            start=(cb == 0),
                            stop=(cb == CB - 1),
                        )
                    cp(dst[kb][:, ih * 512 : (ih + 1) * 512], ps)

                return qk_step

            def qk_group(b, kb, cp=None):
                return [
                    qk_one(b, "wq", kb, 0, cp), qk_one(b, "wq", kb, 1, cp),
                    qk_one(b, "wk", kb, 0, cp), qk_one(b, "wk", kb, 1, cp),
                ]

            def v_steps(b, nbs):
                """v projection, natural [n, (h, ones|d)] into VT[b]."""
                steps = []
                for nb in nbs:

                    def v_step(nb=nb):
                        ps = mmout.tile([P, C], F32, tag="mm", name=f"ps_v_{b}_{nb}")
                        h, loc = nb // 4, nb % 4
                        for cb in range(CB):
                            nc.tensor.matmul(
                                ps,
                                state[b]["xT"][:, h, cb, loc * P : (loc + 1) * P],
                                wsb["wv"][:, cb, :],
                                start=(cb == 0),
                                stop=(cb == CB - 1),
                            )
                        nc.vector.tensor_copy(
                            VT[b][nb][:, :, D : 2 * D],
                            ps.rearrange("p (h d) -> p h d", h=H),
                        )

                    steps.append(v_step)
                return steps

            # --- attention machinery -------------------------------------
            def get_aT(b, hp):
                st = state[b]
                aT = st.setdefault("aT", {})
                if hp not in aT:
                    aT[hp] = big.tile([P, N], BF16, tag=f"aT{hp}", name=f"aT{hp}_b{b}")
                return aT[hp]

            def make_iter(b, hp, ih):
                """Allocate the PSUM accumulators + closures for one
                (head-pair, query-half) iteration."""
                get_aT(b, hp)
                avA = avp.tile([P, 512], F32, tag="av", name=f"avA_{b}_{hp}_{ih}")
                avB = avp.tile([P, 512], F32, tag="av", name=f"avB_{b}_{hp}_{ih}")
                sts = {}
                pts = {}

                def st_step(m):
                    kTt = state[b]["kT"][hp]
                    qTt = state[b]["qT"][hp]
                    isl = slice(ih * 512, (ih + 1) * 512)
                    msl = slice(m * P, (m + 1) * P)
                    st2 = stp.tile([P, 1024], F32, tag="st", name=f"st_{b}_{hp}_{ih}_{m}")
                    sts[m] = st2
                    nc.tensor.matmul(
                        st2[:, 0:512], kTt[0:D, msl], qTt[0:D, isl],
                        start=True, stop=True,
                    )
                    nc.tensor.matmul(
                        st2[:, 512:1024], kTt[D : 2 * D, msl],
                        qTt[D : 2 * D, isl], start=True, stop=True,
                    )

                def exp_step(m):
                    pt2 = ptp.tile([P, 1024], BF16, tag="pt", name=f"pt_{b}_{hp}_{ih}_{m}")
                    pts[m] = pt2
                    nc.scalar.activation(pt2, sts.pop(m), EXP, scale=SCALE)

                def pv_step(m):
                    pt2 = pts.pop(m)
                    nc.tensor.matmul(
                        avA, VT[b][m][:, 2 * hp, :], pt2[:, 0:512],
                        start=(m == 0), stop=(m == NB - 1),
                    )
                    nc.tensor.matmul(
                        avB, VT[b][m][:, 2 * hp + 1, :], pt2[:, 512:1024],
                        start=(m == 0), stop=(m == NB - 1),
                    )

                def norm_step():
                    # approx reciprocal: ~18 correct bits, ~5x faster than
                    # the exact DVE reciprocal.  Denominators sit at PSUM
                    # partitions 0-63 (ones block is first in v tiles).
                    isl = slice(ih * 512, (ih + 1) * 512)
                    aTt = state[b]["aT"][hp]
                    rA = rpool.tile([D, 512], F32, tag="recip", name=f"rA_{b}_{hp}_{ih}")
                    rB = rpool.tile([D, 512], F32, tag="recip", name=f"rB_{b}_{hp}_{ih}")
                    nc.vector.reciprocal_approx_fast(out=rA, in_=avA[0:D, :])
                    nc.vector.tensor_mul(aTt[0:D, isl], avA[D : 2 * D, :], rA)
                    nc.vector.reciprocal_approx_fast(out=rB, in_=avB[0:D, :])
                    nc.vector.tensor_mul(aTt[D : 2 * D, isl], avB[D : 2 * D, :], rB)

                return st_step, exp_step, pv_step, norm_step

            def attention_steps(iter_list):
                """Unified emission for a sequence of iterations across
                batches.  iter_list: [(b, hp, ih, defer), ...].  9 main
                steps per iteration.  Two scheduling tricks keep the ACT
                exp stream gap-free:
                  - the NEXT iteration's ST(0) is hoisted into this
                    iteration's m==7 step, BEFORE PV(7) and any fills, so
                    exp(it+1, 0) can start the moment exp(it, 7) ends;
                  - PV(0..1) are emitted after ST(2), so their wait on the
                    previous iteration's norm (which frees the PSUM
                    accumulators) never delays an ST.
                A deferred iteration emits no PVs until after exp(7) - used
                for b0's first iteration whose v tiles (wv on the slow
                SWDGE ring) arrive mid-iteration."""
                its = [make_iter(b, hp, ih) for (b, hp, ih, _) in iter_list]
                steps = []
                for k, ((b, hp, ih, defer), it) in enumerate(zip(iter_list, its)):
                    st_s, exp_s, pv_s, norm_s = it
                    first = (k == 0)
                    nxt_st = its[k + 1][0] if k + 1 < len(its) else None
                    for m in range(NB):
                        def step(m=m, st_s=st_s, exp_s=exp_s, pv_s=pv_s,
                                 defer=defer, first=first, nxt_st=nxt_st):
                            if m > 0 or first:
                                st_s(m)
                            exp_s(m)
                            if not defer and m >= 2:
                                # PVs trail their exp by two steps, so the
                                # in-order PE queue never waits on an exp
                                # completion (a wait = a pipeline-drain
                                # burst break, ~160ns each, and PE is the
                                # bottleneck engine).
                                pv_s(m - 2)
                            if m == NB - 1 and nxt_st is not None:
                                nxt_st(0)
                        steps.append(step)

                    def tail(defer=defer, pv_s=pv_s, norm_s=norm_s):
                        if defer:
                            for m in range(NB):
                                pv_s(m)
                        else:
                            pv_s(NB - 2)
                            pv_s(NB - 1)
                        norm_s()
                    steps.append(tail)
                return steps

            def proj_steps(b, nbs):
                """One step per output tile: 4 matmuls + bias + store on the
                sync HWDGE ring (engine idle; the gpsimd SWDGE drain cost
                ~3us of tail in v2)."""
                steps = []
                for nb in nbs:

                    def p_step(nb=nb):
                        ps = mmout.tile([P, C], F32, tag="mm", name=f"ps_y_{b}_{nb}")
                        for cb in range(CB):
                            nc.tensor.matmul(
                                ps,
                                state[b]["aT"][cb][:, nb * P : (nb + 1) * P],
                                wsb["wp"][:, cb, :],
                                start=(cb == 0),
                                stop=(cb == CB - 1),
                            )
                        ytile = ypool.tile([P, C], F32, tag="yt", name=f"yt_{b}_{nb}")
                        nc.vector.tensor_add(ytile, ps, bias_bc)
                        nc.sync.dma_start(
                            out=y[b, nb * P : (nb + 1) * P, :], in_=ytile
                        )

                    steps.append(p_step)
                return steps

            def run_interleaved(main_steps, fill_specs):
                """Emit main_steps with fills (step, deadline, not_before)
                distributed evenly, subject to: fill j MUST be emitted
                before main[deadline] (producers have to precede their
                consumers in the per-engine emission order or the consumer
                reads the previous run's stale buffer contents - no
                dependency is created on a not-yet-emitted producer), and
                MUST NOT be emitted before main[not_before] (the reverse
                hazard: a fill that READS data must follow its producers).
                Deadlines must be non-decreasing in list order."""
                main = list(main_steps)
                fills = list(fill_specs)
                nf = len(fills)
                done = 0
                for i, s in enumerate(main):
                    while done < nf and fills[done][1] <= i:
                        fills[done][0]()
                        done += 1
                    s()
                    while done < nf and fills[done][2] <= i + 1:
                        fills[done][0]()
                        done += 1
                while done < nf:
                    fills[done][0]()
                    done += 1

            # --- emission schedule ---------------------------------------
            # Serial prologue: q/k for head-pair 0, query/token half 0 only
            # (x2T.h0 + xT.h0 land ~14us; copies on the idle ACT engine).
            # The remaining input halves are released ring-sequentially as
            # the critical chunks are consumed.
            qk_one(0, "wq", 0, 0, cp=nc.scalar.copy)()
            dma_half(0, "xT", 1, nc.sync, gate=state[0]["qT"][0][0:1, 0:2])
            qk_one(0, "wk", 0, 0, cp=nc.scalar.copy)()
            dma_half(0, "x2T", 1, nc.sync, gate=state[0]["kT"][0][0:1, 0:2])

            # All 16 iterations in one interleave: b0 hp-outer (iteration 1
            # deferred behind wv's slow arrival), then b1 ih0 x 4 hp, then
            # b1 ih1 x 4 hp.  Iteration k spans mains 9k..9k+8; the st0 of
            # iteration k+1 is emitted inside main 9k+7.
            iter_list = [(0, hp, ih, hp == 0 and ih == 0)
                         for hp in range(CB) for ih in range(IH)]
            iter_list += [(1, hp, 0, False) for hp in range(CB)]
            iter_list += [(1, hp, 1, False) for hp in range(CB)]

            b1gate = VT[0][NB - 1][0:1, 0, D : D + 2]
            vs0 = v_steps(0, range(NB))
            vs1 = v_steps(1, range(NB))
            pj0 = proj_steps(0, range(NB))
            pj1 = proj_steps(1, range(NB))

            # Fill specs (step, deadline, target): targets are explicit main
            # indices chosen so each phase's fill load matches its spare PE
            # capacity (~3.3 fill steps per iteration window); deadlines
            # are the emission-order correctness bounds.
            F = []
            F.append((qk_one(0, "wk", 0, 1), 4, 1))
            F.append((vs0[0], 6, 2))
            F.append((vs0[1], 6, 3))
            F.append((qk_one(0, "wq", 0, 1), 7, 4))
            for j, m in enumerate(range(2, NB)):
                F.append((vs0[m], 8, 4 + j // 2))
            for wh, h in (("x2T", 0), ("x2T", 1), ("xT", 0), ("xT", 1)):
                F.append((lambda wh=wh, h=h:
                          dma_half(1, wh, h, nc.sync, gate=b1gate), 16, 8))
            F += [(s, 16, 10 + j) for j, s in enumerate(qk_group(0, 1))]
            F += [(lambda nb=nb: vt_memset(1, nb), 33, 14 + nb) for nb in range(4)]
            F += [(s, 33, 19 + 2 * j) for j, s in enumerate(qk_group(0, 2))]
            F += [(lambda nb=nb: vt_memset(1, nb), 51, 22 + nb) for nb in range(4, NB)]
            F += [(s, 51, 31 + 2 * j) for j, s in enumerate(qk_group(0, 3))]
            F += [(s, 69, 39 + 2 * j) for j, s in enumerate(qk_group(1, 0))]
            F += [(vs1[m], 73, 47 + 2 * m) for m in range(4)]
            F += [(s, 78, [55, 58, 61, 64][j]) for j, s in enumerate(qk_group(1, 1))]
            F += [(vs1[m], [78, 79, 80, 80][m - 4], [66, 68, 73, 75][m - 4])
                  for m in range(4, NB)]
            F += [(s, 87, [78, 81, 84, 86][j]) for j, s in enumerate(qk_group(1, 2))]
            F += [(s, 96, 89 + 2 * j) for j, s in enumerate(qk_group(1, 3))]
            F += [(pj0[j], 144, 109 + 3 * j) for j in range(6)]
            F.append((pj1[0], 144, 126))
            F.append((pj0[6], 144, 127))
            F.append((pj1[1], 144, 130))
            F.append((pj0[7], 144, 130))
            F.append((pj1[2], 144, 134))
            F.append((pj1[3], 144, 138))
            run_interleaved(attention_steps(iter_list), F)

            # Keep the PE busy through the final norm's DVE window (an idle
            # PE can straddle a HAM MID window and re-throttle to 1.2GHz).
            dps2 = mmout.tile([P, 512], F32, tag="mm", name="dps2")
            for i in range(4):
                nc.tensor.matmul(
                    dps2, dummy[:, 0:P], dummy[:, P : P + 512],
                    start=(i == 0), stop=(i == 3),
                )
            for j in range(4, NB):
                pj1[j]()

    nc.compile()
    return nc


def _get_nc():
    if "nc" not in _CACHE:
        _CACHE["nc"] = _build_program()
    return _CACHE["nc"]


def _get_runner():
    """Build (once) a jitted 8-core shard_map executor for the program."""
    if "runner" in _CACHE:
        return _CACHE["runner"]

    import jax
    from jax.experimental.shard_map import shard_map
    from jax.sharding import Mesh, PartitionSpec

    from concourse import bass2jax as b2j

    nc = _get_nc()
    b2j.install_neuronx_cc_hook()
    assert nc.dbg_addr is None
    partition_name = nc.partition_id_tensor.name if nc.partition_id_tensor else None

    in_names = []
    out_names = []
    out_avals = []
    zero_outs = []
    for alloc in nc.m.functions[0].allocations:
        if not isinstance(alloc, mybir.MemoryLocationSet):
            continue
        name = alloc.memorylocations[0].name
        if alloc.kind == "ExternalInput":
            if name != partition_name:
                in_names.append(name)
        elif alloc.kind == "ExternalOutput":
            out_names.append(name)
            shape = tuple(alloc.tensor_shape)
            dtype = mybir.dt.np(alloc.dtype)
            out_avals.append(jax.core.ShapedArray(shape, dtype))
            zero_outs.append(np.zeros(shape, dtype))
    n_params = len(in_names)
    all_names = in_names + out_names
    if partition_name is not None:
        all_names = all_names + [partition_name]

    def _body(*args):
        operands = list(args)
        if partition_name is not None:
            operands.append(b2j.partition_id_tensor())
        outs = b2j._bass_exec_p.bind(
            *operands,
            out_avals=tuple(out_avals),
            in_names=tuple(all_names),
            out_names=tuple(out_names),
            lowering_input_output_aliases=(),
            sim_require_finite=True,
            sim_require_nnan=True,
            nc=nc,
        )
        return tuple(outs)

    devices = jax.devices()[:NCORES]
    mesh = Mesh(np.asarray(devices), ("core",))
    n_outs = len(out_names)
    sharded = jax.jit(
        shard_map(
            _body,
            mesh=mesh,
            in_specs=(PartitionSpec("core"),) * (n_params + n_outs),
            out_specs=(PartitionSpec("core"),) * n_outs,
            check_rep=False,
        ),
        donate_argnums=tuple(range(n_params, n_params + n_outs)),
        keep_unused=True,
    )

    def run(in_maps):
        concat_in = [
            np.concatenate([np.asarray(m[name]) for m in in_maps], axis=0)
            for name in in_names
        ]
        concat_zeros = [
            np.zeros((NCORES * z.shape[0], *z.shape[1:]), z.dtype) for z in zero_outs
        ]
        out_arrs = sharded(*concat_in, *concat_zeros)
        return [
            {
                name: np.asarray(out_arrs[i]).reshape(NCORES, *out_avals[i].shape)[c]
                for i, name in enumerate(out_names)
            }
            for c in range(NCORES)
        ]

    _CACHE["runner_parts"] = dict(
        sharded=sharded,
        in_names=in_names,
        out_names=out_names,
        out_avals=out_avals,
        zero_outs=zero_outs,
        mesh=mesh,
    )
    _CACHE["runner"] = run
    return run


def make_in_maps(x, x2, Wq, Wk, Wv, Wp, bp):
    """Host-side prep shared by kernel() and test harnesses: shard the
    batch; pre-transpose x/x2 to [C, N] bf16, partition-major, split into
    two contiguous 512-token halves [IH, P, CB, 512]; weights pre-
    transposed and arranged [P, CB, C]."""
    import ml_dtypes

    bf16 = ml_dtypes.bfloat16

    def arrange_x(a):
        # [B, N, C] -> [B, C, N] -> [B, CB, P, IH, 512] -> [B, IH, P, CB, 512]
        a = np.asarray(a, dtype=np.float32).astype(bf16).transpose(0, 2, 1)
        a = a.reshape(a.shape[0], CB, P, IH, 512)
        return np.ascontiguousarray(a.transpose(0, 3, 2, 1, 4))

    def arrange_w(w):
        # W [C, C] -> W.T -> [CB, P, C] -> [P, CB, C]
        wt = np.asarray(w, dtype=np.float32).T.astype(bf16)
        return np.ascontiguousarray(wt.reshape(CB, P, C).transpose(1, 0, 2))

    xt = arrange_x(x)
    x2t = arrange_x(x2)
    wqt = arrange_w(Wq)
    wkt = arrange_w(Wk)
    wvt = arrange_w(Wv)
    wpt = arrange_w(Wp)
    bp = np.asarray(bp, dtype=np.float32)

    in_maps = []
    for c in range(NCORES):
        in_maps.append(
            {
                "xts": xt[c * B_LOC : (c + 1) * B_LOC],
                "x2ts": x2t[c * B_LOC : (c + 1) * B_LOC],
                "wqt": wqt,
                "wkt": wkt,
                "wvt": wvt,
                "wpt": wpt,
                "bp": bp,
            }
        )
    return in_maps


def kernel(x, x2, Wq, Wk, Wv, Wp, bp):
    in_maps = make_in_maps(x, x2, Wq, Wk, Wv, Wp, bp)
    if os.environ.get("KERNEL_RUNNER", "cached") == "spmd":
        res = run_bass_kernel_spmd(_get_nc(), in_maps, core_ids=list(range(NCORES)))
        results = res.results
    else:
        run = _get_runner()
        results = run(in_maps)
    out = np.concatenate([r["y"] for r in results], axis=0)
    return out.astype(np.float32)


# revision 20
# speedup vs baseline: 1.0014x; 1.0014x over previous
"""Trainium2 Bass kernel for nn_Attention3D_fusion (cross-attention block).

Reference computation (B=16, N=1024, C=512, H=8, D=64):
    q = (x2 @ Wq.T) -> [B,H,N,D]  (queries from x2)
    k = (x  @ Wk.T) -> [B,H,N,D]
    v = (x  @ Wv.T) -> [B,H,N,D]
    attn = softmax(q @ k.T * D**-0.5)
    out  = (attn @ v) merged heads -> [B,N,C]
    y    = out @ Wp.T + bp
Sharding: batch data-parallel across 8 NeuronCores (2 batches/core), weights
replicated, no collectives.

Per-core kernel strategy (v3):
  - Inputs arrive host-side pre-transposed to [C, N], bf16, partition-major,
    and split into two contiguous 512-token halves [IH, P, CB, 512] so each
    half DMAs as one fully sequential 0.5MB read.
  - Engine budget per core (trace-derived): ACT does only exp, 128 tiles x
    1.11us = 142us; PE slots = attention 96us + projections 55us.  Both are
    at their rooflines; the kernel's job is overlap: wall ~= first-exp time
    + max(ACT chain, PE work) + drain.
  - DMA: two HWDGE rings (sync + scalar engines) at ~150GB/s each carry all
    critical loads, interleaved so q/k projections for head-pair 0 can start
    ~14us; wv + bias ride the slow gpsimd SWDGE ring, which forces the first
    attention iteration's PV matmuls to be deferred until after its exps
    (their results just accumulate later - the exp stream doesn't wait).
    b1's inputs follow on the rings with no gating (FIFO after b0's), and
    all 16 y-tile stores go out on the sync ring (engine otherwise idle) -
    the gpsimd SWDGE drain was 3us of tail in v2.
  - Scores are computed transposed with the two heads of a pair row-packed;
    softmax denominators come free as PV-output rows 0..63 via a 64-wide
    ones block in the v tiles; softmax skips max-subtraction (scores
    ~N(0,0.33), exp cannot overflow).
  - Fill pacing is demand-aware: b0's attention hides b0's remaining
    projections + all of b1's q/k/v prologue; b1's attention hides b0's
    output projection; only y(b1, second half) drains after the last exp.

Measured v2 (8 cores, NTFF): 211.5us.  v3 target ~185us.
"""

import os
import sys

import numpy as np

for _p in ("/opt/trn_rl_repo", "/root/.axon_site/_ro/trn_rl_repo"):
    if os.path.isdir(_p) and _p not in sys.path:
        sys.path.insert(0, _p)

import concourse.bass as bass
import concourse.tile as tile
from concourse import bacc, mybir
from concourse.bass_utils import run_bass_kernel_spmd

B, N, C = 16, 1024, 512
H, D = 8, 64
P = 128
NCORES = 8
B_LOC = B // NCORES  # batches per core
NB = N // P          # 8 token blocks
CB = C // P          # 4 channel blocks (also head-pairs: one block = 2 heads)
IH = N // 512        # 2 query/token halves of 512
SCALE = float(D) ** -0.5
F32 = mybir.dt.float32
BF16 = mybir.dt.bfloat16
EXP = mybir.ActivationFunctionType.Exp

_CACHE = {}


def _build_program():
    nc = bacc.Bacc("TRN2", target_bir_lowering=False, debug=False)

    # Inputs pre-transposed to [C, N] bf16 and arranged token-half-major
    # [IH, P, CB, 512]: each half is one contiguous 0.5MB block with 4KB
    # per-partition lines -> full-rate sequential DRAM reads, and the two
    # halves can ride different DGE rings concurrently.
    xts = nc.dram_tensor("xts", (B_LOC, IH, P, CB, 512), BF16, kind="ExternalInput").ap()
    x2ts = nc.dram_tensor("x2ts", (B_LOC, IH, P, CB, 512), BF16, kind="ExternalInput").ap()
    wqt = nc.dram_tensor("wqt", (P, CB, C), BF16, kind="ExternalInput").ap()
    wkt = nc.dram_tensor("wkt", (P, CB, C), BF16, kind="ExternalInput").ap()
    wvt = nc.dram_tensor("wvt", (P, CB, C), BF16, kind="ExternalInput").ap()
    wpt = nc.dram_tensor("wpt", (P, CB, C), BF16, kind="ExternalInput").ap()
    bp = nc.dram_tensor("bp", (C,), F32, kind="ExternalInput").ap()
    y = nc.dram_tensor("y", (B_LOC, N, C), F32, kind="ExternalOutput").ap()

    with tile.TileContext(nc) as tc:
        with (
            tc.tile_pool(name="consts", bufs=1) as consts,
            tc.tile_pool(name="big", bufs=2) as big,
            tc.tile_pool(name="ptp", bufs=9) as ptp,
            tc.tile_pool(name="ypool", bufs=3) as ypool,
            tc.tile_pool(name="rpool", bufs=4) as rpool,
            tc.tile_pool(name="mmout", bufs=2, space="PSUM") as mmout,
            tc.tile_pool(name="stp", bufs=2, space="PSUM") as stp,
            tc.tile_pool(name="avp", bufs=2, space="PSUM") as avp,
        ):
            # Pre-warm the ACT exp table (~2.7us ACT_TABLE_LOAD) before any
            # scores exist, so the first real exp doesn't pay it.
            warm = consts.tile([1, 16], F32, tag="warm", name="warm")
            nc.vector.memset(warm, 0.0)

            dummy = consts.tile([P, 640], BF16, tag="dummy", name="dummy")
            nc.vector.memset(dummy, 0.125)

            # Weight SBUF tiles.
            wsb = {
                name: consts.tile([P, CB, C], BF16, tag=f"w_{name}", name=f"w_{name}")
                for name in ("wq", "wk", "wv", "wp")
            }

            # Input tiles [P, IH, CB, 512] per batch, loaded as two
            # half-tensor DMAs each.
            state = {}

            def in_tile(b, which):
                st = state.setdefault(b, {})
                if which not in st:
                    st[which] = big.tile(
                        [P, IH, CB, 512], BF16, tag=which, name=f"{which}_b{b}"
                    )
                return st[which]

            def dma_half(b, which, h, eng, gate=None):
                """Load one token half.  `gate`: a produced 2-element
                region; a corner copy from it into the destination makes
                the DMA trigger wait - the SDMA engines round-robin across
                ALL in-flight transfers on a ring, so an ungated transfer
                steals bandwidth from the critical lead-in set."""
                src = x2ts if which == "x2T" else xts
                t = in_tile(b, which)
                if gate is not None:
                    nc.vector.tensor_copy(t[0:1, h, 0, 0:2], gate)
                eng.dma_start(out=t[:, h], in_=src[b, h])

            # --- DMA plan.  Triggers are the first user instructions on
            # each ring so descriptors hit the queues the moment the
            # preamble barrier clears.  Critical wave (ungated): the four
            # 0.5MB chunks the first q/k projections need, two per HWDGE
            # ring so all land ~14us.  Everything else is corner-gated
            # (sync ring only - a gated trigger on the scalar ring would
            # block the exp stream) or rides the slow gpsimd SWDGE.
            nc.sync.dma_start(out=wsb["wq"], in_=wqt)
            nc.scalar.dma_start(out=wsb["wk"], in_=wkt)
            dma_half(0, "x2T", 0, nc.sync)
            dma_half(0, "xT", 0, nc.scalar)
            nc.gpsimd.dma_start(out=wsb["wv"], in_=wvt)

            bias_bc = consts.tile([P, C], F32, tag="bias_bc", name="bias_bc")
            nc.gpsimd.dma_start(
                out=bias_bc,
                in_=bass.AP(tensor=bp.tensor, offset=bp.offset, ap=[[0, P], [1, C]]),
            )
            nc.gpsimd.dma_start(out=wsb["wp"], in_=wpt)

            # ACT exp-table warm (scalar engine, after its dma triggers).
            warm2 = consts.tile([1, 16], F32, tag="warm2", name="warm2")
            nc.scalar.activation(warm2, warm, EXP, scale=SCALE)

            # Persistent per-(batch, token-block) v tiles [P, H, ones|d].
            VT = {
                b: [
                    consts.tile(
                        [P, H, 2 * D], BF16, tag=f"VT{b}_{nb}", name=f"VT{b}_{nb}"
                    )
                    for nb in range(NB)
                ]
                for b in range(B_LOC)
            }

            def vt_memset(b, nb):
                nc.vector.memset(VT[b][nb][:, :, 0:D], 1.0)

            for nb in range(NB):
                vt_memset(0, nb)

            # Dummy-matmul bridge: keeps the PE HAM activity window filled
            # from preamble end (~3.6us) to the first real projection
            # (~14us at the cold 1.2GHz clock), so the clock flips to full
            # rate right as attention begins.
            # 30 dummies: ~8.4us at the cold clock until the HAM flip takes
            # effect (~12us), then ~220ns each at full rate - ends ~15.5us,
            # just as the first critical input chunks land.  Any idle gap
            # here re-throttles the PE to 1.2GHz right as the first real
            # projections start.
            dps = mmout.tile([P, 512], F32, tag="mm", name="dps")
            for i in range(30):
                nc.tensor.matmul(
                    dps, dummy[:, 0:P], dummy[:, P : P + 512],
                    start=(i == 0), stop=(i == 29),
                )

            def qk_one(b, wname, kb, ih, cp=None):
                """One q/k projection step: [P, 512] of transposed output."""
                if cp is None:
                    cp = nc.vector.tensor_copy
                st = state.setdefault(b, {})
                kind = "qT" if wname == "wq" else "kT"
                skey = "x2T" if wname == "wq" else "xT"
                dst = st.setdefault(kind, {})
                if kb not in dst:
                    dst[kb] = big.tile(
                        [P, N], BF16, tag=f"{kind}{kb}", name=f"{kind}{kb}_b{b}"
                    )

                def qk_step():
                    srcT = state[b][skey]
                    ps = mmout.tile(
                        [P, 512], F32, tag="mm", name=f"ps_{kind}_{b}_{kb}_{ih}"
                    )
                    for cb in range(CB):
                        nc.tensor.matmul(
                            ps,
                            wsb[wname][:, cb, kb * P : (kb + 1) * P],
                            srcT[:, ih, cb, :],
                            start=(cb == 0),
                            stop=(cb == CB - 1),
                        )
                    cp(dst[kb][:, ih * 512 : (ih + 1) * 512], ps)

                return qk_step

            def qk_group(b, kb, cp=None):
                return [
                    qk_one(b, "wq", kb, 0, cp), qk_one(b, "wq", kb, 1, cp),
                    qk_one(b, "wk", kb, 0, cp), qk_one(b, "wk", kb, 1, cp),
                ]

            def v_steps(b, nbs):
                """v projection, natural [n, (h, ones|d)] into VT[b]."""
                steps = []
                for nb in nbs:

                    def v_step(nb=nb):
                        ps = mmout.tile([P, C], F32, tag="mm", name=f"ps_v_{b}_{nb}")
                        h, loc = nb // 4, nb % 4
                        for cb in range(CB):
                            nc.tensor.matmul(
                                ps,
                                state[b]["xT"][:, h, cb, loc * P : (loc + 1) * P],
                                wsb["wv"][:, cb, :],
                                start=(cb == 0),
                                stop=(cb == CB - 1),
                            )
                        nc.vector.tensor_copy(
                            VT[b][nb][:, :, D : 2 * D],
                            ps.rearrange("p (h d) -> p h d", h=H),
                        )

                    steps.append(v_step)
                return steps

            # --- attention machinery -------------------------------------
            def get_aT(b, hp):
                st = state[b]
                aT = st.setdefault("aT", {})
                if hp not in aT:
                    aT[hp] = big.tile([P, N], BF16, tag=f"aT{hp}", name=f"aT{hp}_b{b}")
                return aT[hp]

            def make_iter(b, hp, ih):
                """Allocate the PSUM accumulators + closures for one
                (head-pair, query-half) iteration."""
                get_aT(b, hp)
                avA = avp.tile([P, 512], F32, tag="av", name=f"avA_{b}_{hp}_{ih}")
                avB = avp.tile([P, 512], F32, tag="av", name=f"avB_{b}_{hp}_{ih}")
                sts = {}
                pts = {}

                def st_step(m):
                    kTt = state[b]["kT"][hp]
                    qTt = state[b]["qT"][hp]
                    isl = slice(ih * 512, (ih + 1) * 512)
                    msl = slice(m * P, (m + 1) * P)
                    st2 = stp.tile([P, 1024], F32, tag="st", name=f"st_{b}_{hp}_{ih}_{m}")
                    sts[m] = st2
                    nc.tensor.matmul(
                        st2[:, 0:512], kTt[0:D, msl], qTt[0:D, isl],
                        start=True, stop=True,
                    )
                    nc.tensor.matmul(
                        st2[:, 512:1024], kTt[D : 2 * D, msl],
                        qTt[D : 2 * D, isl], start=True, stop=True,
                    )

                def exp_step(m):
                    pt2 = ptp.tile([P, 1024], BF16, tag="pt", name=f"pt_{b}_{hp}_{ih}_{m}")
                    pts[m] = pt2
                    nc.scalar.activation(pt2, sts.pop(m), EXP, scale=SCALE)

                def pv_step(m):
                    pt2 = pts.pop(m)
                    nc.tensor.matmul(
                        avA, VT[b][m][:, 2 * hp, :], pt2[:, 0:512],
                        start=(m == 0), stop=(m == NB - 1),
                    )
                    nc.tensor.matmul(
                        avB, VT[b][m][:, 2 * hp + 1, :], pt2[:, 512:1024],
                        start=(m == 0), stop=(m == NB - 1),
                    )

                def norm_step():
                    # approx reciprocal: ~18 correct bits, ~5x faster than
                    # the exact DVE reciprocal.  Denominators sit at PSUM
                    # partitions 0-63 (ones block is first in v tiles).
                    isl = slice(ih * 512, (ih + 1) * 512)
                    aTt = state[b]["aT"][hp]
                    rA = rpool.tile([D, 512], F32, tag="recip", name=f"rA_{b}_{hp}_{ih}")
                    rB = rpool.tile([D, 512], F32, tag="recip", name=f"rB_{b}_{hp}_{ih}")
                    nc.vector.reciprocal_approx_fast(out=rA, in_=avA[0:D, :])
                    nc.vector.tensor_mul(aTt[0:D, isl], avA[D : 2 * D, :], rA)
                    nc.vector.reciprocal_approx_fast(out=rB, in_=avB[0:D, :])
                    nc.vector.tensor_mul(aTt[D : 2 * D, isl], avB[D : 2 * D, :], rB)

                return st_step, exp_step, pv_step, norm_step

            def attention_steps(iter_list):
                """Unified emission for a sequence of iterations across
                batches.  iter_list: [(b, hp, ih, defer), ...].  9 main
                steps per iteration.  Two scheduling tricks keep the ACT
                exp stream gap-free:
                  - the NEXT iteration's ST(0) is hoisted into this
                    iteration's m==7 step, BEFORE PV(7) and any fills, so
                    exp(it+1, 0) can start the moment exp(it, 7) ends;
                  - PV(0..1) are emitted after ST(2), so their wait on the
                    previous iteration's norm (which frees the PSUM
                    accumulators) never delays an ST.
                A deferred iteration emits no PVs until after exp(7) - used
                for b0's first iteration whose v tiles (wv on the slow
                SWDGE ring) arrive mid-iteration."""
                its = [make_iter(b, hp, ih) for (b, hp, ih, _) in iter_list]
                steps = []
                for k, ((b, hp, ih, defer), it) in enumerate(zip(iter_list, its)):
                    st_s, exp_s, pv_s, norm_s = it
                    first = (k == 0)
                    nxt_st = its[k + 1][0] if k + 1 < len(its) else None
                    for m in range(NB):
                        def step(m=m, st_s=st_s, exp_s=exp_s, pv_s=pv_s,
                                 defer=defer, first=first, nxt_st=nxt_st):
                            if m > 0 or first:
                                st_s(m)
                            exp_s(m)
                            if not defer and m >= 2:
                                # PVs trail their exp by two steps, so the
                                # in-order PE queue never waits on an exp
                                # completion (a wait = a pipeline-drain
                                # burst break, ~160ns each, and PE is the
                                # bottleneck engine).
                                pv_s(m - 2)
                            if m == NB - 1 and nxt_st is not None:
                                nxt_st(0)
                        steps.append(step)

                    def tail(defer=defer, pv_s=pv_s, norm_s=norm_s):
                        if defer:
                            for m in range(NB):
                                pv_s(m)
                        else:
                            pv_s(NB - 2)
                            pv_s(NB - 1)
                        norm_s()
                    steps.append(tail)
                return steps

            def proj_steps(b, nbs):
                """One step per output tile: 4 matmuls + bias + store on the
                sync HWDGE ring (engine idle; the gpsimd SWDGE drain cost
                ~3us of tail in v2)."""
                steps = []
                for nb in nbs:

                    def p_step(nb=nb):
                        ps = mmout.tile([P, C], F32, tag="mm", name=f"ps_y_{b}_{nb}")
                        for cb in range(CB):
                            nc.tensor.matmul(
                                ps,
                                state[b]["aT"][cb][:, nb * P : (nb + 1) * P],
                                wsb["wp"][:, cb, :],
                                start=(cb == 0),
                                stop=(cb == CB - 1),
                            )
                        ytile = ypool.tile([P, C], F32, tag="yt", name=f"yt_{b}_{nb}")
                        nc.vector.tensor_add(ytile, ps, bias_bc)
                        nc.sync.dma_start(
                            out=y[b, nb * P : (nb + 1) * P, :], in_=ytile
                        )

                    steps.append(p_step)
                return steps

            def run_interleaved(main_steps, fill_specs):
                """Emit main_steps with fills (step, deadline, not_before)
                distributed evenly, subject to: fill j MUST be emitted
                before main[deadline] (producers have to precede their
                consumers in the per-engine emission order or the consumer
                reads the previous run's stale buffer contents - no
                dependency is created on a not-yet-emitted producer), and
                MUST NOT be emitted before main[not_before] (the reverse
                hazard: a fill that READS data must follow its producers).
                Deadlines must be non-decreasing in list order."""
                main = list(main_steps)
                fills = list(fill_specs)
                nf = len(fills)
                done = 0
                for i, s in enumerate(main):
                    while done < nf and fills[done][1] <= i:
                        fills[done][0]()
                        done += 1
                    s()
                    while done < nf and fills[done][2] <= i + 1:
                        fills[done][0]()
                        done += 1
                while done < nf:
                    fills[done][0]()
                    done += 1

            # --- emission schedule ---------------------------------------
            # Serial prologue: q/k for head-pair 0, query/token half 0 only
            # (x2T.h0 + xT.h0 land ~14us; copies on the idle ACT engine).
            # The remaining input halves are released ring-sequentially as
            # the critical chunks are consumed.
            qk_one(0, "wq", 0, 0, cp=nc.scalar.copy)()
            dma_half(0, "xT", 1, nc.sync, gate=state[0]["qT"][0][0:1, 0:2])
            qk_one(0, "wk", 0, 0, cp=nc.scalar.copy)()
            dma_half(0, "x2T", 1, nc.sync, gate=state[0]["kT"][0][0:1, 0:2])

            # All 16 iterations in one interleave: b0 hp-outer (iteration 1
            # deferred behind wv's slow arrival), then b1 ih0 x 4 hp, then
            # b1 ih1 x 4 hp.  Iteration k spans mains 9k..9k+8; the st0 of
            # iteration k+1 is emitted inside main 9k+7.
            iter_list = [(0, hp, ih, hp == 0 and ih == 0)
                         for hp in range(CB) for ih in range(IH)]
            iter_list += [(1, hp, 0, False) for hp in range(CB)]
            iter_list += [(1, hp, 1, False) for hp in range(CB)]

            b1gate = VT[0][NB - 1][0:1, 0, D : D + 2]
            vs0 = v_steps(0, range(NB))
            vs1 = v_steps(1, range(NB))
            pj0 = proj_steps(0, range(NB))
            pj1 = proj_steps(1, range(NB))

            # Fill specs (step, deadline, target): targets are explicit main
            # indices chosen so each phase's fill load matches its spare PE
            # capacity (~3.3 fill steps per iteration window); deadlines
            # are the emission-order correctness bounds.
            F = []
            F.append((qk_one(0, "wk", 0, 1), 4, 1))
            F.append((vs0[0], 6, 2))
            F.append((vs0[1], 6, 3))
            F.append((qk_one(0, "wq", 0, 1), 7, 4))
            for j, m in enumerate(range(2, NB)):
                F.append((vs0[m], 8, 4 + j // 2))
            for wh, h in (("x2T", 0), ("x2T", 1), ("xT", 0), ("xT", 1)):
                F.append((lambda wh=wh, h=h:
                          dma_half(1, wh, h, nc.sync, gate=b1gate), 16, 8))
            F += [(s, 16, 10 + j) for j, s in enumerate(qk_group(0, 1))]
            F += [(lambda nb=nb: vt_memset(1, nb), 33, 14 + nb) for nb in range(4)]
            F += [(s, 33, 19 + 2 * j) for j, s in enumerate(qk_group(0, 2))]
            F += [(lambda nb=nb: vt_memset(1, nb), 51, 22 + nb) for nb in range(4, NB)]
            F += [(s, 51, 31 + 2 * j) for j, s in enumerate(qk_group(0, 3))]
            F += [(s, 69, 39 + 2 * j) for j, s in enumerate(qk_group(1, 0))]
            F += [(vs1[m], 73, 47 + 2 * m) for m in range(4)]
            F += [(s, 78, [55, 58, 61, 64][j]) for j, s in enumerate(qk_group(1, 1))]
            F += [(vs1[m], [78, 79, 80, 80][m - 4], [66, 68, 73, 75][m - 4])
                  for m in range(4, NB)]
            F += [(s, 87, [78, 81, 84, 86][j]) for j, s in enumerate(qk_group(1, 2))]
            F += [(s, 96, 89 + 2 * j) for j, s in enumerate(qk_group(1, 3))]
            F += [(pj0[j], 144, 109 + 3 * j) for j in range(6)]
            F.append((pj1[0], 144, 126))
            F.append((pj0[6], 144, 127))
            F.append((pj1[1], 144, 130))
            F.append((pj0[7], 144, 130))
            F.append((pj1[2], 144, 134))
            F.append((pj1[3], 144, 138))
            run_interleaved(attention_steps(iter_list), F)

            # Keep the PE busy through the final norm's ~2.7us DVE window
            # (an idle PE can straddle a HAM MID window and re-throttle to
            # 1.2GHz, making the projection tail run cold).
            dps2 = mmout.tile([P, 512], F32, tag="mm", name="dps2")
            for i in range(12):
                nc.tensor.matmul(
                    dps2, dummy[:, 0:P], dummy[:, P : P + 512],
                    start=(i == 0), stop=(i == 11),
                )
            for j in range(4, NB):
                pj1[j]()

    nc.compile()
    return nc


def _get_nc():
    if "nc" not in _CACHE:
        _CACHE["nc"] = _build_program()
    return _CACHE["nc"]


def _get_runner():
    """Build (once) a jitted 8-core shard_map executor for the program."""
    if "runner" in _CACHE:
        return _CACHE["runner"]

    import jax
    from jax.experimental.shard_map import shard_map
    from jax.sharding import Mesh, PartitionSpec

    from concourse import bass2jax as b2j

    nc = _get_nc()
    b2j.install_neuronx_cc_hook()
    assert nc.dbg_addr is None
    partition_name = nc.partition_id_tensor.name if nc.partition_id_tensor else None

    in_names = []
    out_names = []
    out_avals = []
    zero_outs = []
    for alloc in nc.m.functions[0].allocations:
        if not isinstance(alloc, mybir.MemoryLocationSet):
            continue
        name = alloc.memorylocations[0].name
        if alloc.kind == "ExternalInput":
            if name != partition_name:
                in_names.append(name)
        elif alloc.kind == "ExternalOutput":
            out_names.append(name)
            shape = tuple(alloc.tensor_shape)
            dtype = mybir.dt.np(alloc.dtype)
            out_avals.append(jax.core.ShapedArray(shape, dtype))
            zero_outs.append(np.zeros(shape, dtype))
    n_params = len(in_names)
    all_names = in_names + out_names
    if partition_name is not None:
        all_names = all_names + [partition_name]

    def _body(*args):
        operands = list(args)
        if partition_name is not None:
            operands.append(b2j.partition_id_tensor())
        outs = b2j._bass_exec_p.bind(
            *operands,
            out_avals=tuple(out_avals),
            in_names=tuple(all_names),
            out_names=tuple(out_names),
            lowering_input_output_aliases=(),
            sim_require_finite=True,
            sim_require_nnan=True,
            nc=nc,
        )
        return tuple(outs)

    devices = jax.devices()[:NCORES]
    mesh = Mesh(np.asarray(devices), ("core",))
    n_outs = len(out_names)
    sharded = jax.jit(
        shard_map(
            _body,
            mesh=mesh,
            in_specs=(PartitionSpec("core"),) * (n_params + n_outs),
            out_specs=(PartitionSpec("core"),) * n_outs,
            check_rep=False,
        ),
        donate_argnums=tuple(range(n_params, n_params + n_outs)),
        keep_unused=True,
    )

    def run(in_maps):
        concat_in = [
            np.concatenate([np.asarray(m[name]) for m in in_maps], axis=0)
            for name in in_names
        ]
        concat_zeros = [
            np.zeros((NCORES * z.shape[0], *z.shape[1:]), z.dtype) for z in zero_outs
        ]
        out_arrs = sharded(*concat_in, *concat_zeros)
        return [
            {
                name: np.asarray(out_arrs[i]).reshape(NCORES, *out_avals[i].shape)[c]
                for i, name in enumerate(out_names)
            }
            for c in range(NCORES)
        ]

    _CACHE["runner_parts"] = dict(
        sharded=sharded,
        in_names=in_names,
        out_names=out_names,
        out_avals=out_avals,
        zero_outs=zero_outs,
        mesh=mesh,
    )
    _CACHE["runner"] = run
    return run


def make_in_maps(x, x2, Wq, Wk, Wv, Wp, bp):
    """Host-side prep shared by kernel() and test harnesses: shard the
    batch; pre-transpose x/x2 to [C, N] bf16, partition-major, split into
    two contiguous 512-token halves [IH, P, CB, 512]; weights pre-
    transposed and arranged [P, CB, C]."""
    import ml_dtypes

    bf16 = ml_dtypes.bfloat16

    def arrange_x(a):
        # [B, N, C] -> [B, C, N] -> [B, CB, P, IH, 512] -> [B, IH, P, CB, 512]
        a = np.asarray(a, dtype=np.float32).astype(bf16).transpose(0, 2, 1)
        a = a.reshape(a.shape[0], CB, P, IH, 512)
        return np.ascontiguousarray(a.transpose(0, 3, 2, 1, 4))

    def arrange_w(w):
        # W [C, C] -> W.T -> [CB, P, C] -> [P, CB, C]
        wt = np.asarray(w, dtype=np.float32).T.astype(bf16)
        return np.ascontiguousarray(wt.reshape(CB, P, C).transpose(1, 0, 2))

    xt = arrange_x(x)
    x2t = arrange_x(x2)
    wqt = arrange_w(Wq)
    wkt = arrange_w(Wk)
    wvt = arrange_w(Wv)
    wpt = arrange_w(Wp)
    bp = np.asarray(bp, dtype=np.float32)

    in_maps = []
    for c in range(NCORES):
        in_maps.append(
            {
                "xts": xt[c * B_LOC : (c + 1) * B_LOC],
                "x2ts": x2t[c * B_LOC : (c + 1) * B_LOC],
                "wqt": wqt,
                "wkt": wkt,
                "wvt": wvt,
                "wpt": wpt,
                "bp": bp,
            }
        )
    return in_maps


def kernel(x, x2, Wq, Wk, Wv, Wp, bp):
    in_maps = make_in_maps(x, x2, Wq, Wk, Wv, Wp, bp)
    if os.environ.get("KERNEL_RUNNER", "cached") == "spmd":
        res = run_bass_kernel_spmd(_get_nc(), in_maps, core_ids=list(range(NCORES)))
        results = res.results
    else:
        run = _get_runner()
        results = run(in_maps)
    out = np.concatenate([r["y"] for r in results], axis=0)
    return out.astype(np.float32)


# revision 28
# speedup vs baseline: 1.0624x; 1.0609x over previous
"""Trainium2 Bass kernel for nn_Attention3D_fusion (cross-attention block).

Reference computation (B=16, N=1024, C=512, H=8, D=64):
    q = (x2 @ Wq.T) -> [B,H,N,D]  (queries from x2)
    k = (x  @ Wk.T) -> [B,H,N,D]
    v = (x  @ Wv.T) -> [B,H,N,D]
    attn = softmax(q @ k.T * D**-0.5)
    out  = (attn @ v) merged heads -> [B,N,C]
    y    = out @ Wp.T + bp
Sharding: batch data-parallel across 8 NeuronCores (2 batches/core), weights
replicated, no collectives.

Per-core kernel strategy (v3):
  - Inputs arrive host-side pre-transposed to [C, N], bf16, partition-major,
    and split into two contiguous 512-token halves [IH, P, CB, 512] so each
    half DMAs as one fully sequential 0.5MB read.
  - Engine budget per core (trace-derived): ACT does only exp, 128 tiles x
    1.11us = 142us; PE slots = attention 96us + projections 55us.  Both are
    at their rooflines; the kernel's job is overlap: wall ~= first-exp time
    + max(ACT chain, PE work) + drain.
  - DMA: two HWDGE rings (sync + scalar engines) at ~150GB/s each carry all
    critical loads, interleaved so q/k projections for head-pair 0 can start
    ~14us; wv + bias ride the slow gpsimd SWDGE ring, which forces the first
    attention iteration's PV matmuls to be deferred until after its exps
    (their results just accumulate later - the exp stream doesn't wait).
    b1's inputs follow on the rings with no gating (FIFO after b0's), and
    all 16 y-tile stores go out on the sync ring (engine otherwise idle) -
    the gpsimd SWDGE drain was 3us of tail in v2.
  - Scores are computed transposed with the two heads of a pair row-packed;
    softmax denominators come free as PV-output rows 0..63 via a 64-wide
    ones block in the v tiles; softmax skips max-subtraction (scores
    ~N(0,0.33), exp cannot overflow).
  - Fill pacing is demand-aware: b0's attention hides b0's remaining
    projections + all of b1's q/k/v prologue; b1's attention hides b0's
    output projection; only y(b1, second half) drains after the last exp.

Measured v2 (8 cores, NTFF): 211.5us.  v3 target ~185us.
"""

import os
import sys

import numpy as np

for _p in ("/opt/trn_rl_repo", "/root/.axon_site/_ro/trn_rl_repo"):
    if os.path.isdir(_p) and _p not in sys.path:
        sys.path.insert(0, _p)

import concourse.bass as bass
import concourse.tile as tile
from concourse import bacc, mybir
from concourse.bass_utils import run_bass_kernel_spmd

B, N, C = 16, 1024, 512
H, D = 8, 64
P = 128
NCORES = 8
B_LOC = B // NCORES  # batches per core
NB = N // P          # 8 token blocks
CB = C // P          # 4 channel blocks (also head-pairs: one block = 2 heads)
IH = N // 512        # 2 query/token halves of 512
SCALE = float(D) ** -0.5
F32 = mybir.dt.float32
BF16 = mybir.dt.bfloat16
FP8 = mybir.dt.float8e4
EXP = mybir.ActivationFunctionType.Exp
DR = mybir.MatmulPerfMode.DoubleRow

_CACHE = {}


def _build_program():
    nc = bacc.Bacc("TRN2", target_bir_lowering=False, debug=False)

    # Inputs pre-transposed to [C, N] bf16 and arranged token-half-major
    # [IH, P, CB, 512]: each half is one contiguous 0.5MB block with 4KB
    # per-partition lines -> full-rate sequential DRAM reads, and the two
    # halves can ride different DGE rings concurrently.
    xts = nc.dram_tensor("xts", (B_LOC, IH, P, CB, 512), BF16, kind="ExternalInput").ap()
    x2ts = nc.dram_tensor("x2ts", (B_LOC, IH, P, CB, 512), BF16, kind="ExternalInput").ap()
    wqt = nc.dram_tensor("wqt", (P, CB, C), BF16, kind="ExternalInput").ap()
    wkt = nc.dram_tensor("wkt", (P, CB, C), BF16, kind="ExternalInput").ap()
    wvt = nc.dram_tensor("wvt", (P, CB, C), BF16, kind="ExternalInput").ap()
    wpt = nc.dram_tensor("wpt", (P, CB, C), BF16, kind="ExternalInput").ap()
    bp = nc.dram_tensor("bp", (C,), F32, kind="ExternalInput").ap()
    y = nc.dram_tensor("y", (B_LOC, N, C), F32, kind="ExternalOutput").ap()

    with tile.TileContext(nc) as tc:
        with (
            tc.tile_pool(name="consts", bufs=1) as consts,
            tc.tile_pool(name="big", bufs=2) as big,
            tc.tile_pool(name="ptp", bufs=9) as ptp,
            tc.tile_pool(name="ypool", bufs=3) as ypool,
            tc.tile_pool(name="rpool", bufs=4) as rpool,
            tc.tile_pool(name="mmout", bufs=2, space="PSUM") as mmout,
            tc.tile_pool(name="stp", bufs=2, space="PSUM") as stp,
            tc.tile_pool(name="avp", bufs=2, space="PSUM") as avp,
        ):
            # Pre-warm the ACT exp table (~2.7us ACT_TABLE_LOAD) before any
            # scores exist, so the first real exp doesn't pay it.
            warm = consts.tile([1, 16], F32, tag="warm", name="warm")
            nc.vector.memset(warm, 0.0)

            dummy = consts.tile([P, 640], BF16, tag="dummy", name="dummy")
            nc.vector.memset(dummy, 0.125)

            # Weight SBUF tiles.
            wsb = {
                name: consts.tile([P, CB, C], BF16, tag=f"w_{name}", name=f"w_{name}")
                for name in ("wq", "wk", "wv", "wp")
            }

            # Input tiles [P, IH, CB, 512] per batch, loaded as two
            # half-tensor DMAs each.
            state = {}

            def in_tile(b, which):
                st = state.setdefault(b, {})
                if which not in st:
                    st[which] = big.tile(
                        [P, IH, CB, 512], BF16, tag=which, name=f"{which}_b{b}"
                    )
                return st[which]

            def dma_half(b, which, h, eng, gate=None):
                """Load one token half.  `gate`: a produced 2-element
                region; a corner copy from it into the destination makes
                the DMA trigger wait - the SDMA engines round-robin across
                ALL in-flight transfers on a ring, so an ungated transfer
                steals bandwidth from the critical lead-in set."""
                src = x2ts if which == "x2T" else xts
                t = in_tile(b, which)
                if gate is not None:
                    nc.vector.tensor_copy(t[0:1, h, 0, 0:2], gate)
                eng.dma_start(out=t[:, h], in_=src[b, h])

            # --- DMA plan.  Triggers are the first user instructions on
            # each ring so descriptors hit the queues the moment the
            # preamble barrier clears.  Critical wave (ungated): the four
            # 0.5MB chunks the first q/k projections need, two per HWDGE
            # ring so all land ~14us.  Everything else is corner-gated
            # (sync ring only - a gated trigger on the scalar ring would
            # block the exp stream) or rides the slow gpsimd SWDGE.
            nc.sync.dma_start(out=wsb["wq"], in_=wqt)
            nc.scalar.dma_start(out=wsb["wk"], in_=wkt)
            dma_half(0, "x2T", 0, nc.sync)
            dma_half(0, "xT", 0, nc.scalar)
            nc.gpsimd.dma_start(out=wsb["wv"], in_=wvt)

            bias_bc = consts.tile([P, C], F32, tag="bias_bc", name="bias_bc")
            nc.gpsimd.dma_start(
                out=bias_bc,
                in_=bass.AP(tensor=bp.tensor, offset=bp.offset, ap=[[0, P], [1, C]]),
            )
            nc.gpsimd.dma_start(out=wsb["wp"], in_=wpt)

            # ACT exp-table warm (scalar engine, after its dma triggers).
            warm2 = consts.tile([1, 16], F32, tag="warm2", name="warm2")
            nc.scalar.activation(warm2, warm, EXP, scale=SCALE)

            # Persistent per-(batch, token-block-PAIR) v tiles in fp8:
            # [P, 2 (m-subtile), H, ones|d].  Two m-blocks share a tile so
            # one fp8 DoubleRow matmul contracts 256 keys at 2x rate.  The
            # ones blocks still provide softmax denominators for free.
            VT = {
                b: [
                    consts.tile(
                        [P, 2, H, 2 * D], FP8, tag=f"VT{b}_{mp}", name=f"VT{b}_{mp}"
                    )
                    for mp in range(NB // 2)
                ]
                for b in range(B_LOC)
            }

            def vt_memset(b, mp):
                nc.vector.memset(VT[b][mp][:, :, :, 0:D], 1.0)

            for mp in range(NB // 2):
                vt_memset(0, mp)

            # Dummy-matmul bridge: keeps the PE HAM activity window filled
            # from preamble end (~3.6us) to the first real projection
            # (~14us at the cold 1.2GHz clock), so the clock flips to full
            # rate right as attention begins.
            # 30 dummies: ~8.4us at the cold clock until the HAM flip takes
            # effect (~12us), then ~220ns each at full rate - ends ~15.5us,
            # just as the first critical input chunks land.  Any idle gap
            # here re-throttles the PE to 1.2GHz right as the first real
            # projections start.
            dps = mmout.tile([P, 512], F32, tag="mm", name="dps")
            for i in range(30):
                nc.tensor.matmul(
                    dps, dummy[:, 0:P], dummy[:, P : P + 512],
                    start=(i == 0), stop=(i == 29),
                )

            def qk_one(b, wname, kb, ih, cp=None):
                """One q/k projection step: [P, 512] of transposed output."""
                if cp is None:
                    cp = nc.vector.tensor_copy
                st = state.setdefault(b, {})
                kind = "qT" if wname == "wq" else "kT"
                skey = "x2T" if wname == "wq" else "xT"
                dst = st.setdefault(kind, {})
                if kb not in dst:
                    dst[kb] = big.tile(
                        [P, N], BF16, tag=f"{kind}{kb}", name=f"{kind}{kb}_b{b}"
                    )

                def qk_step():
                    srcT = state[b][skey]
                    ps = mmout.tile(
                        [P, 512], F32, tag="mm", name=f"ps_{kind}_{b}_{kb}_{ih}"
                    )
                    for cb in range(CB):
                        nc.tensor.matmul(
                            ps,
                            wsb[wname][:, cb, kb * P : (kb + 1) * P],
                            srcT[:, ih, cb, :],
                            start=(cb == 0),
                            stop=(cb == CB - 1),
                        )
                    cp(dst[kb][:, ih * 512 : (ih + 1) * 512], ps)

                return qk_step

            def qk_group(b, kb, cp=None):
                return [
                    qk_one(b, "wq", kb, 0, cp), qk_one(b, "wq", kb, 1, cp),
                    qk_one(b, "wk", kb, 0, cp), qk_one(b, "wk", kb, 1, cp),
                ]

            def v_steps(b, nbs):
                """v projection, natural [n, (h, ones|d)] into VT[b]."""
                steps = []
                for nb in nbs:

                    def v_step(nb=nb):
                        ps = mmout.tile([P, C], F32, tag="mm", name=f"ps_v_{b}_{nb}")
                        h, loc = nb // 4, nb % 4
                        for cb in range(CB):
                            nc.tensor.matmul(
                                ps,
                                state[b]["xT"][:, h, cb, loc * P : (loc + 1) * P],
                                wsb["wv"][:, cb, :],
                                start=(cb == 0),
                                stop=(cb == CB - 1),
                            )
                        nc.vector.tensor_copy(
                            VT[b][nb // 2][:, nb % 2, :, D : 2 * D],
                            ps.rearrange("p (h d) -> p h d", h=H),
                        )

                    steps.append(v_step)
                return steps

            # --- attention machinery -------------------------------------
            def get_aT(b, hp):
                st = state[b]
                aT = st.setdefault("aT", {})
                if hp not in aT:
                    aT[hp] = big.tile([P, N], BF16, tag=f"aT{hp}", name=f"aT{hp}_b{b}")
                return aT[hp]

            def make_iter(b, hp, ih):
                """Allocate the PSUM accumulators + closures for one
                (head-pair, query-half) iteration."""
                get_aT(b, hp)
                avA = avp.tile([P, 512], F32, tag="av", name=f"avA_{b}_{hp}_{ih}")
                avB = avp.tile([P, 512], F32, tag="av", name=f"avB_{b}_{hp}_{ih}")
                sts = {}
                pts = {}

                def st_step(m):
                    kTt = state[b]["kT"][hp]
                    qTt = state[b]["qT"][hp]
                    isl = slice(ih * 512, (ih + 1) * 512)
                    msl = slice(m * P, (m + 1) * P)
                    st2 = stp.tile([P, 1024], F32, tag="st", name=f"st_{b}_{hp}_{ih}_{m}")
                    sts[m] = st2
                    nc.tensor.matmul(
                        st2[:, 0:512], kTt[0:D, msl], qTt[0:D, isl],
                        start=True, stop=True,
                    )
                    nc.tensor.matmul(
                        st2[:, 512:1024], kTt[D : 2 * D, msl],
                        qTt[D : 2 * D, isl], start=True, stop=True,
                    )

                def exp_step(m):
                    # exp writes fp8e4 directly into one m-subtile of a
                    # [P, 2, 1024] pair tile (values in [~0.2, 5] - well
                    # inside e4m3 range).
                    mp = m // 2
                    if m % 2 == 0:
                        pts[mp] = ptp.tile(
                            [P, 2, 1024], FP8, tag="pt", name=f"pt_{b}_{hp}_{ih}_{mp}"
                        )
                    nc.scalar.activation(pts[mp][:, m % 2, :], sts.pop(m), EXP,
                                         scale=SCALE)

                def pv_step(mp):
                    # fp8 DoubleRow: one matmul contracts both m-subtiles
                    # (256 keys) at 2x rate - halves the PV slot count on
                    # the bottleneck PE.
                    pt2 = pts.pop(mp)
                    nc.tensor.matmul(
                        avA, VT[b][mp][:, :, 2 * hp, :], pt2[:, :, 0:512],
                        start=(mp == 0), stop=(mp == NB // 2 - 1), perf_mode=DR,
                    )
                    nc.tensor.matmul(
                        avB, VT[b][mp][:, :, 2 * hp + 1, :], pt2[:, :, 512:1024],
                        start=(mp == 0), stop=(mp == NB // 2 - 1), perf_mode=DR,
                    )

                def norm_step():
                    # approx reciprocal: ~18 correct bits, ~5x faster than
                    # the exact DVE reciprocal.  Denominators sit at PSUM
                    # partitions 0-63 (ones block is first in v tiles).
                    isl = slice(ih * 512, (ih + 1) * 512)
                    aTt = state[b]["aT"][hp]
                    rA = rpool.tile([D, 512], F32, tag="recip", name=f"rA_{b}_{hp}_{ih}")
                    rB = rpool.tile([D, 512], F32, tag="recip", name=f"rB_{b}_{hp}_{ih}")
                    nc.vector.reciprocal_approx_fast(out=rA, in_=avA[0:D, :])
                    nc.vector.tensor_mul(aTt[0:D, isl], avA[D : 2 * D, :], rA)
                    nc.vector.reciprocal_approx_fast(out=rB, in_=avB[0:D, :])
                    nc.vector.tensor_mul(aTt[D : 2 * D, isl], avB[D : 2 * D, :], rB)

                return st_step, exp_step, pv_step, norm_step

            def attention_steps(iter_list):
                """Unified emission for a sequence of iterations across
                batches.  iter_list: [(b, hp, ih, defer), ...].  9 main
                steps per iteration.  Two scheduling tricks keep the ACT
                exp stream gap-free:
                  - the NEXT iteration's ST(0) is hoisted into this
                    iteration's m==7 step, BEFORE PV(7) and any fills, so
                    exp(it+1, 0) can start the moment exp(it, 7) ends;
                  - PV(0..1) are emitted after ST(2), so their wait on the
                    previous iteration's norm (which frees the PSUM
                    accumulators) never delays an ST.
                A deferred iteration emits no PVs until after exp(7) - used
                for b0's first iteration whose v tiles (wv on the slow
                SWDGE ring) arrive mid-iteration."""
                its = [make_iter(b, hp, ih) for (b, hp, ih, _) in iter_list]
                steps = []
                for k, ((b, hp, ih, defer), it) in enumerate(zip(iter_list, its)):
                    st_s, exp_s, pv_s, norm_s = it
                    first = (k == 0)
                    nxt_st = its[k + 1][0] if k + 1 < len(its) else None
                    for m in range(NB):
                        def step(m=m, st_s=st_s, exp_s=exp_s, pv_s=pv_s,
                                 defer=defer, first=first, nxt_st=nxt_st):
                            if m > 0 or first:
                                st_s(m)
                            exp_s(m)
                            if m == NB - 1 and nxt_st is not None:
                                nxt_st(0)
                            # PV(mp) trails the second exp of its m-pair by
                            # two steps, so the in-order PE queue never
                            # waits on an exp completion (a wait = a
                            # pipeline-drain burst break, ~160ns each, and
                            # PE is the bottleneck engine).
                            if not defer and m >= 3 and m % 2 == 1:
                                pv_s((m - 3) // 2)
                        steps.append(step)

                    def tail(defer=defer, pv_s=pv_s, norm_s=norm_s):
                        if defer:
                            for mp in range(NB // 2 - 1):
                                pv_s(mp)
                        pv_s(NB // 2 - 1)
                        norm_s()
                    steps.append(tail)
                return steps

            def proj_steps(b, nbs):
                """One step per output tile: 4 matmuls + bias + store on the
                sync HWDGE ring (engine idle; the gpsimd SWDGE drain cost
                ~3us of tail in v2)."""
                steps = []
                for nb in nbs:

                    def p_step(nb=nb):
                        ps = mmout.tile([P, C], F32, tag="mm", name=f"ps_y_{b}_{nb}")
                        for cb in range(CB):
                            nc.tensor.matmul(
                                ps,
                                state[b]["aT"][cb][:, nb * P : (nb + 1) * P],
                                wsb["wp"][:, cb, :],
                                start=(cb == 0),
                                stop=(cb == CB - 1),
                            )
                        ytile = ypool.tile([P, C], F32, tag="yt", name=f"yt_{b}_{nb}")
                        nc.vector.tensor_add(ytile, ps, bias_bc)
                        nc.sync.dma_start(
                            out=y[b, nb * P : (nb + 1) * P, :], in_=ytile
                        )

                    steps.append(p_step)
                return steps

            def run_interleaved(main_steps, fill_specs):
                """Emit main_steps with fills (step, deadline, not_before)
                distributed evenly, subject to: fill j MUST be emitted
                before main[deadline] (producers have to precede their
                consumers in the per-engine emission order or the consumer
                reads the previous run's stale buffer contents - no
                dependency is created on a not-yet-emitted producer), and
                MUST NOT be emitted before main[not_before] (the reverse
                hazard: a fill that READS data must follow its producers).
                Deadlines must be non-decreasing in list order."""
                main = list(main_steps)
                fills = list(fill_specs)
                nf = len(fills)
                done = 0
                for i, s in enumerate(main):
                    while done < nf and fills[done][1] <= i:
                        fills[done][0]()
                        done += 1
                    s()
                    while done < nf and fills[done][2] <= i + 1:
                        fills[done][0]()
                        done += 1
                while done < nf:
                    fills[done][0]()
                    done += 1

            # --- emission schedule ---------------------------------------
            # Serial prologue: q/k for head-pair 0, query/token half 0 only
            # (x2T.h0 + xT.h0 land ~14us; copies on the idle ACT engine).
            # The remaining input halves are released ring-sequentially as
            # the critical chunks are consumed.
            qk_one(0, "wq", 0, 0, cp=nc.scalar.copy)()
            dma_half(0, "xT", 1, nc.sync, gate=state[0]["qT"][0][0:1, 0:2])
            qk_one(0, "wk", 0, 0, cp=nc.scalar.copy)()
            dma_half(0, "x2T", 1, nc.sync, gate=state[0]["kT"][0][0:1, 0:2])

            # All 16 iterations in one interleave: b0 hp-outer (iteration 1
            # deferred behind wv's slow arrival), then b1 ih0 x 4 hp, then
            # b1 ih1 x 4 hp.  Iteration k spans mains 9k..9k+8; the st0 of
            # iteration k+1 is emitted inside main 9k+7.
            iter_list = [(0, hp, ih, hp == 0 and ih == 0)
                         for hp in range(CB) for ih in range(IH)]
            iter_list += [(1, hp, 0, False) for hp in range(CB)]
            iter_list += [(1, hp, 1, False) for hp in range(CB)]

            b1gate = VT[0][NB // 2 - 1][0:1, 1, 0, D : D + 2]
            vs0 = v_steps(0, range(NB))
            vs1 = v_steps(1, range(NB))
            pj0 = proj_steps(0, range(NB))
            pj1 = proj_steps(1, range(NB))

            # Fill specs (step, deadline, target): targets are explicit main
            # indices chosen so each phase's fill load matches its spare PE
            # capacity (~3.3 fill steps per iteration window); deadlines
            # are the emission-order correctness bounds.
            F = []
            F.append((qk_one(0, "wk", 0, 1), 4, 1))
            F.append((vs0[0], 6, 2))
            F.append((vs0[1], 6, 3))
            F.append((qk_one(0, "wq", 0, 1), 7, 4))
            for j, m in enumerate(range(2, NB)):
                F.append((vs0[m], 8, 4 + j // 2))
            for wh, h in (("x2T", 0), ("x2T", 1), ("xT", 0), ("xT", 1)):
                F.append((lambda wh=wh, h=h:
                          dma_half(1, wh, h, nc.sync, gate=b1gate), 16, 8))
            F += [(s, 16, 10 + j) for j, s in enumerate(qk_group(0, 1))]
            F += [(lambda mp=mp: vt_memset(1, mp), 33, 14 + mp) for mp in range(2)]
            F += [(s, 33, 19 + 2 * j) for j, s in enumerate(qk_group(0, 2))]
            F += [(lambda mp=mp: vt_memset(1, mp), 51, 22 + mp) for mp in range(2, 4)]
            F += [(s, 51, 31 + 2 * j) for j, s in enumerate(qk_group(0, 3))]
            F += [(s, 69, 39 + 2 * j) for j, s in enumerate(qk_group(1, 0))]
            # v(1, nb) must precede PV(nb//2) of b1's first iteration:
            # PV(mp) is emitted at main 72 + (2*mp + 3), PV(3) in the tail
            # step (main 80).
            F += [(vs1[m], 73, 47 + 2 * m) for m in range(4)]
            F += [(s, 78, [55, 58, 61, 64][j]) for j, s in enumerate(qk_group(1, 1))]
            F += [(vs1[m], [79, 79, 80, 80][m - 4], [66, 68, 73, 75][m - 4])
                  for m in range(4, NB)]
            F += [(s, 87, [78, 81, 84, 86][j]) for j, s in enumerate(qk_group(1, 2))]
            F += [(s, 96, 89 + 2 * j) for j, s in enumerate(qk_group(1, 3))]
            F += [(pj0[j], 144, 109 + 3 * j) for j in range(6)]
            F.append((pj1[0], 144, 126))
            F.append((pj0[6], 144, 127))
            F.append((pj1[1], 144, 130))
            F.append((pj0[7], 144, 130))
            F.append((pj1[2], 144, 134))
            F.append((pj1[3], 144, 138))
            run_interleaved(attention_steps(iter_list), F)

            # Keep the PE busy through the final norm's ~2.7us DVE window
            # (an idle PE can straddle a HAM MID window and re-throttle to
            # 1.2GHz, making the projection tail run cold).
            dps2 = mmout.tile([P, 512], F32, tag="mm", name="dps2")
            for i in range(12):
                nc.tensor.matmul(
                    dps2, dummy[:, 0:P], dummy[:, P : P + 512],
                    start=(i == 0), stop=(i == 11),
                )
            for j in range(4, NB):
                pj1[j]()

    nc.compile()
    return nc


def _get_nc():
    if "nc" not in _CACHE:
        _CACHE["nc"] = _build_program()
    return _CACHE["nc"]


def _get_runner():
    """Build (once) a jitted 8-core shard_map executor for the program."""
    if "runner" in _CACHE:
        return _CACHE["runner"]

    import jax
    from jax.experimental.shard_map import shard_map
    from jax.sharding import Mesh, PartitionSpec

    from concourse import bass2jax as b2j

    nc = _get_nc()
    b2j.install_neuronx_cc_hook()
    assert nc.dbg_addr is None
    partition_name = nc.partition_id_tensor.name if nc.partition_id_tensor else None

    in_names = []
    out_names = []
    out_avals = []
    zero_outs = []
    for alloc in nc.m.functions[0].allocations:
        if not isinstance(alloc, mybir.MemoryLocationSet):
            continue
        name = alloc.memorylocations[0].name
        if alloc.kind == "ExternalInput":
            if name != partition_name:
                in_names.append(name)
        elif alloc.kind == "ExternalOutput":
            out_names.append(name)
            shape = tuple(alloc.tensor_shape)
            dtype = mybir.dt.np(alloc.dtype)
            out_avals.append(jax.core.ShapedArray(shape, dtype))
            zero_outs.append(np.zeros(shape, dtype))
    n_params = len(in_names)
    all_names = in_names + out_names
    if partition_name is not None:
        all_names = all_names + [partition_name]

    def _body(*args):
        operands = list(args)
        if partition_name is not None:
            operands.append(b2j.partition_id_tensor())
        outs = b2j._bass_exec_p.bind(
            *operands,
            out_avals=tuple(out_avals),
            in_names=tuple(all_names),
            out_names=tuple(out_names),
            lowering_input_output_aliases=(),
            sim_require_finite=True,
            sim_require_nnan=True,
            nc=nc,
        )
        return tuple(outs)

    devices = jax.devices()[:NCORES]
    mesh = Mesh(np.asarray(devices), ("core",))
    n_outs = len(out_names)
    sharded = jax.jit(
        shard_map(
            _body,
            mesh=mesh,
            in_specs=(PartitionSpec("core"),) * (n_params + n_outs),
            out_specs=(PartitionSpec("core"),) * n_outs,
            check_rep=False,
        ),
        donate_argnums=tuple(range(n_params, n_params + n_outs)),
        keep_unused=True,
    )

    def run(in_maps):
        concat_in = [
            np.concatenate([np.asarray(m[name]) for m in in_maps], axis=0)
            for name in in_names
        ]
        concat_zeros = [
            np.zeros((NCORES * z.shape[0], *z.shape[1:]), z.dtype) for z in zero_outs
        ]
        out_arrs = sharded(*concat_in, *concat_zeros)
        return [
            {
                name: np.asarray(out_arrs[i]).reshape(NCORES, *out_avals[i].shape)[c]
                for i, name in enumerate(out_names)
            }
            for c in range(NCORES)
        ]

    _CACHE["runner_parts"] = dict(
        sharded=sharded,
        in_names=in_names,
        out_names=out_names,
        out_avals=out_avals,
        zero_outs=zero_outs,
        mesh=mesh,
    )
    _CACHE["runner"] = run
    return run


def make_in_maps(x, x2, Wq, Wk, Wv, Wp, bp):
    """Host-side prep shared by kernel() and test harnesses: shard the
    batch; pre-transpose x/x2 to [C, N] bf16, partition-major, split into
    two contiguous 512-token halves [IH, P, CB, 512]; weights pre-
    transposed and arranged [P, CB, C]."""
    import ml_dtypes

    bf16 = ml_dtypes.bfloat16

    def arrange_x(a):
        # [B, N, C] -> [B, C, N] -> [B, CB, P, IH, 512] -> [B, IH, P, CB, 512]
        a = np.asarray(a, dtype=np.float32).astype(bf16).transpose(0, 2, 1)
        a = a.reshape(a.shape[0], CB, P, IH, 512)
        return np.ascontiguousarray(a.transpose(0, 3, 2, 1, 4))

    def arrange_w(w):
        # W [C, C] -> W.T -> [CB, P, C] -> [P, CB, C]
        wt = np.asarray(w, dtype=np.float32).T.astype(bf16)
        return np.ascontiguousarray(wt.reshape(CB, P, C).transpose(1, 0, 2))

    xt = arrange_x(x)
    x2t = arrange_x(x2)
    wqt = arrange_w(Wq)
    wkt = arrange_w(Wk)
    wvt = arrange_w(Wv)
    wpt = arrange_w(Wp)
    bp = np.asarray(bp, dtype=np.float32)

    in_maps = []
    for c in range(NCORES):
        in_maps.append(
            {
                "xts": xt[c * B_LOC : (c + 1) * B_LOC],
                "x2ts": x2t[c * B_LOC : (c + 1) * B_LOC],
                "wqt": wqt,
                "wkt": wkt,
                "wvt": wvt,
                "wpt": wpt,
                "bp": bp,
            }
        )
    return in_maps


def kernel(x, x2, Wq, Wk, Wv, Wp, bp):
    in_maps = make_in_maps(x, x2, Wq, Wk, Wv, Wp, bp)
    if os.environ.get("KERNEL_RUNNER", "cached") == "spmd":
        res = run_bass_kernel_spmd(_get_nc(), in_maps, core_ids=list(range(NCORES)))
        results = res.results
    else:
        run = _get_runner()
        results = run(in_maps)
    out = np.concatenate([r["y"] for r in results], axis=0)
    return out.astype(np.float32)


# revision 33
# speedup vs baseline: 1.0656x; 1.0030x over previous
"""Trainium2 Bass kernel for nn_Attention3D_fusion (cross-attention block).

Reference computation (B=16, N=1024, C=512, H=8, D=64):
    q = (x2 @ Wq.T) -> [B,H,N,D]  (queries from x2)
    k = (x  @ Wk.T) -> [B,H,N,D]
    v = (x  @ Wv.T) -> [B,H,N,D]
    attn = softmax(q @ k.T * D**-0.5)
    out  = (attn @ v) merged heads -> [B,N,C]
    y    = out @ Wp.T + bp
Sharding: batch data-parallel across 8 NeuronCores (2 batches/core), weights
replicated, no collectives.

Per-core kernel strategy (v3):
  - Inputs arrive host-side pre-transposed to [C, N], bf16, partition-major,
    and split into two contiguous 512-token halves [IH, P, CB, 512] so each
    half DMAs as one fully sequential 0.5MB read.
  - Engine budget per core (trace-derived): ACT does only exp, 128 tiles x
    1.11us = 142us; PE slots = attention 96us + projections 55us.  Both are
    at their rooflines; the kernel's job is overlap: wall ~= first-exp time
    + max(ACT chain, PE work) + drain.
  - DMA: two HWDGE rings (sync + scalar engines) at ~150GB/s each carry all
    critical loads, interleaved so q/k projections for head-pair 0 can start
    ~14us; wv + bias ride the slow gpsimd SWDGE ring, which forces the first
    attention iteration's PV matmuls to be deferred until after its exps
    (their results just accumulate later - the exp stream doesn't wait).
    b1's inputs follow on the rings with no gating (FIFO after b0's), and
    all 16 y-tile stores go out on the sync ring (engine otherwise idle) -
    the gpsimd SWDGE drain was 3us of tail in v2.
  - Scores are computed transposed with the two heads of a pair row-packed;
    softmax denominators come free as PV-output rows 0..63 via a 64-wide
    ones block in the v tiles; softmax skips max-subtraction (scores
    ~N(0,0.33), exp cannot overflow).
  - Fill pacing is demand-aware: b0's attention hides b0's remaining
    projections + all of b1's q/k/v prologue; b1's attention hides b0's
    output projection; only y(b1, second half) drains after the last exp.

Measured v2 (8 cores, NTFF): 211.5us.  v3 target ~185us.
"""

import os
import sys

import numpy as np

for _p in ("/opt/trn_rl_repo", "/root/.axon_site/_ro/trn_rl_repo"):
    if os.path.isdir(_p) and _p not in sys.path:
        sys.path.insert(0, _p)

import concourse.bass as bass
import concourse.tile as tile
from concourse import bacc, mybir
from concourse.bass_utils import run_bass_kernel_spmd

B, N, C = 16, 1024, 512
H, D = 8, 64
P = 128
NCORES = 8
B_LOC = B // NCORES  # batches per core
NB = N // P          # 8 token blocks
CB = C // P          # 4 channel blocks (also head-pairs: one block = 2 heads)
IH = N // 512        # 2 query/token halves of 512
SCALE = float(D) ** -0.5
F32 = mybir.dt.float32
BF16 = mybir.dt.bfloat16
FP8 = mybir.dt.float8e4
EXP = mybir.ActivationFunctionType.Exp
DR = mybir.MatmulPerfMode.DoubleRow

_CACHE = {}


def _build_program():
    nc = bacc.Bacc("TRN2", target_bir_lowering=False, debug=False)

    # Inputs pre-transposed to [C, N] bf16 and arranged token-half-major
    # [IH, P, CB, 512]: each half is one contiguous 0.5MB block with 4KB
    # per-partition lines -> full-rate sequential DRAM reads, and the two
    # halves can ride different DGE rings concurrently.
    xts = nc.dram_tensor("xts", (B_LOC, IH, P, CB, 512), BF16, kind="ExternalInput").ap()
    x2ts = nc.dram_tensor("x2ts", (B_LOC, IH, P, CB, 512), BF16, kind="ExternalInput").ap()
    wqt = nc.dram_tensor("wqt", (P, CB, C), BF16, kind="ExternalInput").ap()
    wkt = nc.dram_tensor("wkt", (P, CB, C), BF16, kind="ExternalInput").ap()
    wvt = nc.dram_tensor("wvt", (P, CB, C), BF16, kind="ExternalInput").ap()
    wpt = nc.dram_tensor("wpt", (P, CB, C), BF16, kind="ExternalInput").ap()
    bp = nc.dram_tensor("bp", (C,), F32, kind="ExternalInput").ap()
    y = nc.dram_tensor("y", (B_LOC, N, C), F32, kind="ExternalOutput").ap()

    with tile.TileContext(nc) as tc:
        with (
            tc.tile_pool(name="consts", bufs=1) as consts,
            tc.tile_pool(name="big", bufs=2) as big,
            tc.tile_pool(name="ptp", bufs=9) as ptp,
            tc.tile_pool(name="ypool", bufs=3) as ypool,
            tc.tile_pool(name="rpool", bufs=4) as rpool,
            tc.tile_pool(name="mmout", bufs=2, space="PSUM") as mmout,
            tc.tile_pool(name="stp", bufs=2, space="PSUM") as stp,
            tc.tile_pool(name="avp", bufs=2, space="PSUM") as avp,
        ):
            # Pre-warm the ACT exp table (~2.7us ACT_TABLE_LOAD) before any
            # scores exist, so the first real exp doesn't pay it.
            warm = consts.tile([1, 16], F32, tag="warm", name="warm")
            nc.vector.memset(warm, 0.0)

            dummy = consts.tile([P, 640], BF16, tag="dummy", name="dummy")
            nc.vector.memset(dummy, 0.125)

            # Weight SBUF tiles.
            wsb = {
                name: consts.tile([P, CB, C], BF16, tag=f"w_{name}", name=f"w_{name}")
                for name in ("wq", "wk", "wv", "wp")
            }

            # Input tiles [P, IH, CB, 512] per batch, loaded as two
            # half-tensor DMAs each.
            state = {}

            def in_tile(b, which):
                st = state.setdefault(b, {})
                if which not in st:
                    st[which] = big.tile(
                        [P, IH, CB, 512], BF16, tag=which, name=f"{which}_b{b}"
                    )
                return st[which]

            def dma_half(b, which, h, eng, gate=None):
                """Load one token half.  `gate`: a produced 2-element
                region; a corner copy from it into the destination makes
                the DMA trigger wait - the SDMA engines round-robin across
                ALL in-flight transfers on a ring, so an ungated transfer
                steals bandwidth from the critical lead-in set."""
                src = x2ts if which == "x2T" else xts
                t = in_tile(b, which)
                if gate is not None:
                    nc.vector.tensor_copy(t[0:1, h, 0, 0:2], gate)
                eng.dma_start(out=t[:, h], in_=src[b, h])

            # --- DMA plan.  Triggers are the first user instructions on
            # each ring so descriptors hit the queues the moment the
            # preamble barrier clears.  Critical wave (ungated): the four
            # 0.5MB chunks the first q/k projections need, two per HWDGE
            # ring so all land ~14us.  Everything else is corner-gated
            # (sync ring only - a gated trigger on the scalar ring would
            # block the exp stream) or rides the slow gpsimd SWDGE.
            nc.sync.dma_start(out=wsb["wq"], in_=wqt)
            nc.scalar.dma_start(out=wsb["wk"], in_=wkt)
            dma_half(0, "x2T", 0, nc.sync)
            dma_half(0, "xT", 0, nc.scalar)
            nc.gpsimd.dma_start(out=wsb["wv"], in_=wvt)

            bias_bc = consts.tile([P, C], F32, tag="bias_bc", name="bias_bc")
            nc.gpsimd.dma_start(
                out=bias_bc,
                in_=bass.AP(tensor=bp.tensor, offset=bp.offset, ap=[[0, P], [1, C]]),
            )
            nc.gpsimd.dma_start(out=wsb["wp"], in_=wpt)

            # ACT exp-table warm (scalar engine, after its dma triggers).
            warm2 = consts.tile([1, 16], F32, tag="warm2", name="warm2")
            nc.scalar.activation(warm2, warm, EXP, scale=SCALE)

            # Persistent per-(batch, token-block-PAIR) v tiles in fp8:
            # [P, 2 (m-subtile), H, ones|d].  Two m-blocks share a tile so
            # one fp8 DoubleRow matmul contracts 256 keys at 2x rate.  The
            # ones blocks still provide softmax denominators for free.
            VT = {
                b: [
                    consts.tile(
                        [P, 2, H, 2 * D], FP8, tag=f"VT{b}_{mp}", name=f"VT{b}_{mp}"
                    )
                    for mp in range(NB // 2)
                ]
                for b in range(B_LOC)
            }

            def vt_memset(b, mp):
                nc.vector.memset(VT[b][mp][:, :, :, 0:D], 1.0)

            for mp in range(NB // 2):
                vt_memset(0, mp)

            # Dummy-matmul bridge: keeps the PE HAM activity window filled
            # from preamble end (~3.6us) to the first real projection
            # (~14us at the cold 1.2GHz clock), so the clock flips to full
            # rate right as attention begins.
            # 30 dummies: ~8.4us at the cold clock until the HAM flip takes
            # effect (~12us), then ~220ns each at full rate - ends ~15.5us,
            # just as the first critical input chunks land.  Any idle gap
            # here re-throttles the PE to 1.2GHz right as the first real
            # projections start.
            dps = mmout.tile([P, 512], F32, tag="mm", name="dps")
            for i in range(30):
                nc.tensor.matmul(
                    dps, dummy[:, 0:P], dummy[:, P : P + 512],
                    start=(i == 0), stop=(i == 29),
                )

            def qk_one(b, wname, kb, ih, cp=None):
                """One q/k projection step: [P, 512] of transposed output."""
                if cp is None:
                    cp = nc.vector.tensor_copy
                st = state.setdefault(b, {})
                kind = "qT" if wname == "wq" else "kT"
                skey = "x2T" if wname == "wq" else "xT"
                dst = st.setdefault(kind, {})
                if kb not in dst:
                    dst[kb] = big.tile(
                        [P, N], BF16, tag=f"{kind}{kb}", name=f"{kind}{kb}_b{b}"
                    )

                def qk_step():
                    srcT = state[b][skey]
                    ps = mmout.tile(
                        [P, 512], F32, tag="mm", name=f"ps_{kind}_{b}_{kb}_{ih}"
                    )
                    for cb in range(CB):
                        nc.tensor.matmul(
                            ps,
                            wsb[wname][:, cb, kb * P : (kb + 1) * P],
                            srcT[:, ih, cb, :],
                            start=(cb == 0),
                            stop=(cb == CB - 1),
                        )
                    cp(dst[kb][:, ih * 512 : (ih + 1) * 512], ps)

                return qk_step

            def qk_group(b, kb, cp=None):
                return [
                    qk_one(b, "wq", kb, 0, cp), qk_one(b, "wq", kb, 1, cp),
                    qk_one(b, "wk", kb, 0, cp), qk_one(b, "wk", kb, 1, cp),
                ]

            def v_steps(b, nbs):
                """v projection, natural [n, (h, ones|d)] into VT[b]."""
                steps = []
                for nb in nbs:

                    def v_step(nb=nb):
                        ps = mmout.tile([P, C], F32, tag="mm", name=f"ps_v_{b}_{nb}")
                        h, loc = nb // 4, nb % 4
                        for cb in range(CB):
                            nc.tensor.matmul(
                                ps,
                                state[b]["xT"][:, h, cb, loc * P : (loc + 1) * P],
                                wsb["wv"][:, cb, :],
                                start=(cb == 0),
                                stop=(cb == CB - 1),
                            )
                        nc.vector.tensor_copy(
                            VT[b][nb // 2][:, nb % 2, :, D : 2 * D],
                            ps.rearrange("p (h d) -> p h d", h=H),
                        )

                    steps.append(v_step)
                return steps

            # --- attention machinery -------------------------------------
            def get_aT(b, hp):
                st = state[b]
                aT = st.setdefault("aT", {})
                if hp not in aT:
                    aT[hp] = big.tile([P, N], BF16, tag=f"aT{hp}", name=f"aT{hp}_b{b}")
                return aT[hp]

            def make_iter(b, hp, ih):
                """Allocate the PSUM accumulators + closures for one
                (head-pair, query-half) iteration."""
                get_aT(b, hp)
                avA = avp.tile([P, 512], F32, tag="av", name=f"avA_{b}_{hp}_{ih}")
                avB = avp.tile([P, 512], F32, tag="av", name=f"avB_{b}_{hp}_{ih}")
                sts = {}
                pts = {}

                def st_step(m):
                    kTt = state[b]["kT"][hp]
                    qTt = state[b]["qT"][hp]
                    isl = slice(ih * 512, (ih + 1) * 512)
                    msl = slice(m * P, (m + 1) * P)
                    st2 = stp.tile([P, 1024], F32, tag="st", name=f"st_{b}_{hp}_{ih}_{m}")
                    sts[m] = st2
                    nc.tensor.matmul(
                        st2[:, 0:512], kTt[0:D, msl], qTt[0:D, isl],
                        start=True, stop=True,
                    )
                    nc.tensor.matmul(
                        st2[:, 512:1024], kTt[D : 2 * D, msl],
                        qTt[D : 2 * D, isl], start=True, stop=True,
                    )

                def exp_step(m):
                    # exp writes fp8e4 directly into one m-subtile of a
                    # [P, 2, 1024] pair tile (values in [~0.2, 5] - well
                    # inside e4m3 range).
                    mp = m // 2
                    if m % 2 == 0:
                        pts[mp] = ptp.tile(
                            [P, 2, 1024], FP8, tag="pt", name=f"pt_{b}_{hp}_{ih}_{mp}"
                        )
                    nc.scalar.activation(pts[mp][:, m % 2, :], sts.pop(m), EXP,
                                         scale=SCALE)

                def pv_step(mp):
                    # fp8 DoubleRow: one matmul contracts both m-subtiles
                    # (256 keys) at 2x rate - halves the PV slot count on
                    # the bottleneck PE.
                    pt2 = pts.pop(mp)
                    nc.tensor.matmul(
                        avA, VT[b][mp][:, :, 2 * hp, :], pt2[:, :, 0:512],
                        start=(mp == 0), stop=(mp == NB // 2 - 1), perf_mode=DR,
                    )
                    nc.tensor.matmul(
                        avB, VT[b][mp][:, :, 2 * hp + 1, :], pt2[:, :, 512:1024],
                        start=(mp == 0), stop=(mp == NB // 2 - 1), perf_mode=DR,
                    )

                rA = rpool.tile([D, 512], F32, tag="recip", name=f"rA_{b}_{hp}_{ih}")
                rB = rpool.tile([D, 512], F32, tag="recip", name=f"rB_{b}_{hp}_{ih}")

                def norm_step():
                    # approx reciprocal: ~18 correct bits, ~5x faster than
                    # the exact DVE reciprocal.  Denominators sit at PSUM
                    # partitions 0-63 (ones block is first in v tiles).
                    isl = slice(ih * 512, (ih + 1) * 512)
                    aTt = state[b]["aT"][hp]
                    nc.vector.reciprocal_approx_fast(out=rA, in_=avA[0:D, :])
                    nc.vector.tensor_mul(aTt[0:D, isl], avA[D : 2 * D, :], rA)
                    nc.vector.reciprocal_approx_fast(out=rB, in_=avB[0:D, :])
                    nc.vector.tensor_mul(aTt[D : 2 * D, isl], avB[D : 2 * D, :], rB)

                def norm_recips():
                    nc.vector.reciprocal_approx_fast(out=rA, in_=avA[0:D, :])
                    nc.vector.reciprocal_approx_fast(out=rB, in_=avB[0:D, :])

                def norm_mul_chunk(j):
                    # one 128-token chunk of the normalization - lets the
                    # final output projections start ~1.5us earlier and
                    # pipeline with the rest of the norm.
                    aTt = state[b]["aT"][hp]
                    csl = slice(j * P, (j + 1) * P)
                    asl = slice(ih * 512 + j * P, ih * 512 + (j + 1) * P)
                    nc.vector.tensor_mul(aTt[0:D, asl], avA[D : 2 * D, csl], rA[:, csl])
                    nc.vector.tensor_mul(aTt[D : 2 * D, asl], avB[D : 2 * D, csl], rB[:, csl])

                return st_step, exp_step, pv_step, norm_step, norm_recips, norm_mul_chunk

            def attention_steps(iter_list):
                """Unified emission for a sequence of iterations across
                batches.  iter_list: [(b, hp, ih, defer), ...].  9 main
                steps per iteration.  Two scheduling tricks keep the ACT
                exp stream gap-free:
                  - the NEXT iteration's ST(0) is hoisted into this
                    iteration's m==7 step, BEFORE PV(7) and any fills, so
                    exp(it+1, 0) can start the moment exp(it, 7) ends;
                  - PV(0..1) are emitted after ST(2), so their wait on the
                    previous iteration's norm (which frees the PSUM
                    accumulators) never delays an ST.
                A deferred iteration emits no PVs until after exp(7) - used
                for b0's first iteration whose v tiles (wv on the slow
                SWDGE ring) arrive mid-iteration."""
                its = [make_iter(b, hp, ih) for (b, hp, ih, _) in iter_list]
                steps = []
                nit = len(iter_list)
                for k, ((b, hp, ih, defer), it) in enumerate(zip(iter_list, its)):
                    st_s, exp_s, pv_s, norm_s, recips_s, mulc_s = it
                    if k == nit - 1:
                        last_handles["mulc"] = mulc_s
                        norm_s = recips_s
                    first = (k == 0)
                    nxt_st = its[k + 1][0] if k + 1 < len(its) else None
                    for m in range(NB):
                        def step(m=m, st_s=st_s, exp_s=exp_s, pv_s=pv_s,
                                 defer=defer, first=first, nxt_st=nxt_st):
                            if m > 0 or first:
                                st_s(m)
                            exp_s(m)
                            if m == NB - 1 and nxt_st is not None:
                                nxt_st(0)
                            # PV(mp) trails the second exp of its m-pair by
                            # two steps, so the in-order PE queue never
                            # waits on an exp completion (a wait = a
                            # pipeline-drain burst break, ~160ns each, and
                            # PE is the bottleneck engine).
                            if not defer and m >= 3 and m % 2 == 1:
                                pv_s((m - 3) // 2)
                        steps.append(step)

                    def tail(defer=defer, pv_s=pv_s, norm_s=norm_s):
                        if defer:
                            for mp in range(NB // 2 - 1):
                                pv_s(mp)
                        pv_s(NB // 2 - 1)
                        norm_s()
                    steps.append(tail)
                return steps

            def proj_steps(b, nbs):
                """One step per output tile: 4 matmuls + bias + store on the
                sync HWDGE ring (engine idle; the gpsimd SWDGE drain cost
                ~3us of tail in v2)."""
                steps = []
                for nb in nbs:

                    def p_step(nb=nb):
                        ps = mmout.tile([P, C], F32, tag="mm", name=f"ps_y_{b}_{nb}")
                        for cb in range(CB):
                            nc.tensor.matmul(
                                ps,
                                state[b]["aT"][cb][:, nb * P : (nb + 1) * P],
                                wsb["wp"][:, cb, :],
                                start=(cb == 0),
                                stop=(cb == CB - 1),
                            )
                        ytile = ypool.tile([P, C], F32, tag="yt", name=f"yt_{b}_{nb}")
                        nc.vector.tensor_add(ytile, ps, bias_bc)
                        nc.sync.dma_start(
                            out=y[b, nb * P : (nb + 1) * P, :], in_=ytile
                        )

                    steps.append(p_step)
                return steps

            def run_interleaved(main_steps, fill_specs):
                """Emit main_steps with fills (step, deadline, not_before)
                distributed evenly, subject to: fill j MUST be emitted
                before main[deadline] (producers have to precede their
                consumers in the per-engine emission order or the consumer
                reads the previous run's stale buffer contents - no
                dependency is created on a not-yet-emitted producer), and
                MUST NOT be emitted before main[not_before] (the reverse
                hazard: a fill that READS data must follow its producers).
                Deadlines must be non-decreasing in list order."""
                main = list(main_steps)
                fills = list(fill_specs)
                nf = len(fills)
                done = 0
                for i, s in enumerate(main):
                    while done < nf and fills[done][1] <= i:
                        fills[done][0]()
                        done += 1
                    s()
                    while done < nf and fills[done][2] <= i + 1:
                        fills[done][0]()
                        done += 1
                while done < nf:
                    fills[done][0]()
                    done += 1

            # --- emission schedule ---------------------------------------
            # The second token halves are gated on the dummy chain's PSUM
            # output - a pure time gate that releases right as the critical
            # chunks finish (~16us), so they never round-robin against them
            # but start 3us earlier than a gate on the first projections
            # would allow.  One per ring so both land ~19.5us.
            dma_half(0, "xT", 1, nc.sync, gate=dps[0:1, 0:2])
            dma_half(0, "x2T", 1, nc.scalar, gate=dps[0:1, 2:4])

            # Serial prologue: q/k for head-pair 0, query/token half 0 only
            # (x2T.h0 + xT.h0 land ~14us; copies on the idle ACT engine).
            qk_one(0, "wq", 0, 0, cp=nc.scalar.copy)()
            qk_one(0, "wk", 0, 0, cp=nc.scalar.copy)()

            # All 16 iterations in one interleave: b0 hp-outer (iteration 1
            # deferred behind wv's slow arrival), then b1 ih0 x 4 hp, then
            # b1 ih1 x 4 hp.  Iteration k spans mains 9k..9k+8; the st0 of
            # iteration k+1 is emitted inside main 9k+7.
            iter_list = [(0, hp, ih, hp == 0 and ih == 0)
                         for hp in range(CB) for ih in range(IH)]
            iter_list += [(1, hp, 0, False) for hp in range(CB)]
            iter_list += [(1, hp, 1, False) for hp in range(CB)]
            last_handles = {}

            b1gate = VT[0][NB // 2 - 1][0:1, 1, 0, D : D + 2]
            vs0 = v_steps(0, range(NB))
            vs1 = v_steps(1, range(NB))
            pj0 = proj_steps(0, range(NB))
            pj1 = proj_steps(1, range(NB))

            # Fill specs (step, deadline, target): targets are explicit main
            # indices chosen so each phase's fill load matches its spare PE
            # capacity (~3.3 fill steps per iteration window); deadlines
            # are the emission-order correctness bounds.
            F = []
            F.append((qk_one(0, "wk", 0, 1), 4, 1))
            F.append((vs0[0], 6, 2))
            F.append((vs0[1], 6, 3))
            F.append((qk_one(0, "wq", 0, 1), 7, 4))
            for j, m in enumerate(range(2, NB)):
                F.append((vs0[m], 8, 4 + j // 2))
            for wh, h in (("x2T", 0), ("x2T", 1), ("xT", 0), ("xT", 1)):
                F.append((lambda wh=wh, h=h:
                          dma_half(1, wh, h, nc.sync, gate=b1gate), 16, 8))
            F += [(s, 16, 10 + j) for j, s in enumerate(qk_group(0, 1))]
            F += [(lambda mp=mp: vt_memset(1, mp), 33, 14 + mp) for mp in range(2)]
            F += [(s, 33, 19 + 2 * j) for j, s in enumerate(qk_group(0, 2))]
            F += [(lambda mp=mp: vt_memset(1, mp), 51, 22 + mp) for mp in range(2, 4)]
            F += [(s, 51, 31 + 2 * j) for j, s in enumerate(qk_group(0, 3))]
            F += [(s, 69, 39 + 2 * j) for j, s in enumerate(qk_group(1, 0))]
            # v(1, nb) must precede PV(nb//2) of b1's first iteration:
            # PV(mp) is emitted at main 72 + (2*mp + 3), PV(3) in the tail
            # step (main 80).
            F += [(vs1[m], 73, 47 + 2 * m) for m in range(4)]
            F += [(s, 78, [55, 58, 61, 64][j]) for j, s in enumerate(qk_group(1, 1))]
            F += [(vs1[m], [79, 79, 80, 80][m - 4], [66, 68, 73, 75][m - 4])
                  for m in range(4, NB)]
            F += [(s, 87, [78, 81, 84, 86][j]) for j, s in enumerate(qk_group(1, 2))]
            F += [(s, 96, 89 + 2 * j) for j, s in enumerate(qk_group(1, 3))]
            F += [(pj0[j], 144, 109 + 3 * j) for j in range(6)]
            F.append((pj1[0], 144, 126))
            F.append((pj0[6], 144, 127))
            F.append((pj1[1], 144, 130))
            F.append((pj0[7], 144, 130))
            F.append((pj1[2], 144, 134))
            F.append((pj1[3], 144, 138))
            run_interleaved(attention_steps(iter_list), F)

            # Keep the PE busy through the final recips' DVE window (an
            # idle PE can straddle a HAM MID window and re-throttle to
            # 1.2GHz, making the projection tail run cold), then drain the
            # last four output tiles chunk-by-chunk behind the norm muls.
            dps2 = mmout.tile([P, 512], F32, tag="mm", name="dps2")
            for i in range(8):
                nc.tensor.matmul(
                    dps2, dummy[:, 0:P], dummy[:, P : P + 512],
                    start=(i == 0), stop=(i == 7),
                )
            for j in range(4):
                last_handles["mulc"](j)
                pj1[4 + j]()

    nc.compile()
    return nc


def _get_nc():
    if "nc" not in _CACHE:
        _CACHE["nc"] = _build_program()
    return _CACHE["nc"]


def _get_runner():
    """Build (once) a jitted 8-core shard_map executor for the program."""
    if "runner" in _CACHE:
        return _CACHE["runner"]

    import jax
    from jax.experimental.shard_map import shard_map
    from jax.sharding import Mesh, PartitionSpec

    from concourse import bass2jax as b2j

    nc = _get_nc()
    b2j.install_neuronx_cc_hook()
    assert nc.dbg_addr is None
    partition_name = nc.partition_id_tensor.name if nc.partition_id_tensor else None

    in_names = []
    out_names = []
    out_avals = []
    zero_outs = []
    for alloc in nc.m.functions[0].allocations:
        if not isinstance(alloc, mybir.MemoryLocationSet):
            continue
        name = alloc.memorylocations[0].name
        if alloc.kind == "ExternalInput":
            if name != partition_name:
                in_names.append(name)
        elif alloc.kind == "ExternalOutput":
            out_names.append(name)
            shape = tuple(alloc.tensor_shape)
            dtype = mybir.dt.np(alloc.dtype)
            out_avals.append(jax.core.ShapedArray(shape, dtype))
            zero_outs.append(np.zeros(shape, dtype))
    n_params = len(in_names)
    all_names = in_names + out_names
    if partition_name is not None:
        all_names = all_names + [partition_name]

    def _body(*args):
        operands = list(args)
        if partition_name is not None:
            operands.append(b2j.partition_id_tensor())
        outs = b2j._bass_exec_p.bind(
            *operands,
            out_avals=tuple(out_avals),
            in_names=tuple(all_names),
            out_names=tuple(out_names),
            lowering_input_output_aliases=(),
            sim_require_finite=True,
            sim_require_nnan=True,
            nc=nc,
        )
        return tuple(outs)

    devices = jax.devices()[:NCORES]
    mesh = Mesh(np.asarray(devices), ("core",))
    n_outs = len(out_names)
    sharded = jax.jit(
        shard_map(
            _body,
            mesh=mesh,
            in_specs=(PartitionSpec("core"),) * (n_params + n_outs),
            out_specs=(PartitionSpec("core"),) * n_outs,
            check_rep=False,
        ),
        donate_argnums=tuple(range(n_params, n_params + n_outs)),
        keep_unused=True,
    )

    def run(in_maps):
        concat_in = [
            np.concatenate([np.asarray(m[name]) for m in in_maps], axis=0)
            for name in in_names
        ]
        concat_zeros = [
            np.zeros((NCORES * z.shape[0], *z.shape[1:]), z.dtype) for z in zero_outs
        ]
        out_arrs = sharded(*concat_in, *concat_zeros)
        return [
            {
                name: np.asarray(out_arrs[i]).reshape(NCORES, *out_avals[i].shape)[c]
                for i, name in enumerate(out_names)
            }
            for c in range(NCORES)
        ]

    _CACHE["runner_parts"] = dict(
        sharded=sharded,
        in_names=in_names,
        out_names=out_names,
        out_avals=out_avals,
        zero_outs=zero_outs,
        mesh=mesh,
    )
    _CACHE["runner"] = run
    return run


def make_in_maps(x, x2, Wq, Wk, Wv, Wp, bp):
    """Host-side prep shared by kernel() and test harnesses: shard the
    batch; pre-transpose x/x2 to [C, N] bf16, partition-major, split into
    two contiguous 512-token halves [IH, P, CB, 512]; weights pre-
    transposed and arranged [P, CB, C]."""
    import ml_dtypes

    bf16 = ml_dtypes.bfloat16

    def arrange_x(a):
        # [B, N, C] -> [B, C, N] -> [B, CB, P, IH, 512] -> [B, IH, P, CB, 512]
        a = np.asarray(a, dtype=np.float32).astype(bf16).transpose(0, 2, 1)
        a = a.reshape(a.shape[0], CB, P, IH, 512)
        return np.ascontiguousarray(a.transpose(0, 3, 2, 1, 4))

    def arrange_w(w):
        # W [C, C] -> W.T -> [CB, P, C] -> [P, CB, C]
        wt = np.asarray(w, dtype=np.float32).T.astype(bf16)
        return np.ascontiguousarray(wt.reshape(CB, P, C).transpose(1, 0, 2))

    xt = arrange_x(x)
    x2t = arrange_x(x2)
    wqt = arrange_w(Wq)
    wkt = arrange_w(Wk)
    wvt = arrange_w(Wv)
    wpt = arrange_w(Wp)
    bp = np.asarray(bp, dtype=np.float32)

    in_maps = []
    for c in range(NCORES):
        in_maps.append(
            {
                "xts": xt[c * B_LOC : (c + 1) * B_LOC],
                "x2ts": x2t[c * B_LOC : (c + 1) * B_LOC],
                "wqt": wqt,
                "wkt": wkt,
                "wvt": wvt,
                "wpt": wpt,
                "bp": bp,
            }
        )
    return in_maps


def kernel(x, x2, Wq, Wk, Wv, Wp, bp):
    in_maps = make_in_maps(x, x2, Wq, Wk, Wv, Wp, bp)
    if os.environ.get("KERNEL_RUNNER", "cached") == "spmd":
        res = run_bass_kernel_spmd(_get_nc(), in_maps, core_ids=list(range(NCORES)))
        results = res.results
    else:
        run = _get_runner()
        results = run(in_maps)
    out = np.concatenate([r["y"] for r in results], axis=0)
    return out.astype(np.float32)


# revision 37
# speedup vs baseline: 1.0779x; 1.0116x over previous
"""Trainium2 Bass kernel for nn_Attention3D_fusion (cross-attention block).

Reference computation (B=16, N=1024, C=512, H=8, D=64):
    q = (x2 @ Wq.T) -> [B,H,N,D]  (queries from x2)
    k = (x  @ Wk.T) -> [B,H,N,D]
    v = (x  @ Wv.T) -> [B,H,N,D]
    attn = softmax(q @ k.T * D**-0.5)
    out  = (attn @ v) merged heads -> [B,N,C]
    y    = out @ Wp.T + bp
Sharding: batch data-parallel across 8 NeuronCores (2 batches/core), weights
replicated, no collectives.

Per-core kernel strategy (v3):
  - Inputs arrive host-side pre-transposed to [C, N], bf16, partition-major,
    and split into two contiguous 512-token halves [IH, P, CB, 512] so each
    half DMAs as one fully sequential 0.5MB read.
  - Engine budget per core (trace-derived): ACT does only exp, 128 tiles x
    1.11us = 142us; PE slots = attention 96us + projections 55us.  Both are
    at their rooflines; the kernel's job is overlap: wall ~= first-exp time
    + max(ACT chain, PE work) + drain.
  - DMA: two HWDGE rings (sync + scalar engines) at ~150GB/s each carry all
    critical loads, interleaved so q/k projections for head-pair 0 can start
    ~14us; wv + bias ride the slow gpsimd SWDGE ring, which forces the first
    attention iteration's PV matmuls to be deferred until after its exps
    (their results just accumulate later - the exp stream doesn't wait).
    b1's inputs follow on the rings with no gating (FIFO after b0's), and
    all 16 y-tile stores go out on the sync ring (engine otherwise idle) -
    the gpsimd SWDGE drain was 3us of tail in v2.
  - Scores are computed transposed with the two heads of a pair row-packed;
    softmax denominators come free as PV-output rows 0..63 via a 64-wide
    ones block in the v tiles; softmax skips max-subtraction (scores
    ~N(0,0.33), exp cannot overflow).
  - Fill pacing is demand-aware: b0's attention hides b0's remaining
    projections + all of b1's q/k/v prologue; b1's attention hides b0's
    output projection; only y(b1, second half) drains after the last exp.

Measured v2 (8 cores, NTFF): 211.5us.  v3 target ~185us.
"""

import os
import sys

import numpy as np

for _p in ("/opt/trn_rl_repo", "/root/.axon_site/_ro/trn_rl_repo"):
    if os.path.isdir(_p) and _p not in sys.path:
        sys.path.insert(0, _p)

import concourse.bass as bass
import concourse.tile as tile
from concourse import bacc, mybir
from concourse.bass_utils import run_bass_kernel_spmd

B, N, C = 16, 1024, 512
H, D = 8, 64
P = 128
NCORES = 8
B_LOC = B // NCORES  # batches per core
NB = N // P          # 8 token blocks
CB = C // P          # 4 channel blocks (also head-pairs: one block = 2 heads)
IH = N // 512        # 2 query/token halves of 512
SCALE = float(D) ** -0.5
F32 = mybir.dt.float32
BF16 = mybir.dt.bfloat16
FP8 = mybir.dt.float8e4
EXP = mybir.ActivationFunctionType.Exp
DR = mybir.MatmulPerfMode.DoubleRow

_CACHE = {}


def _build_program():
    nc = bacc.Bacc("TRN2", target_bir_lowering=False, debug=False)

    # Inputs pre-transposed to [C, N] bf16 and arranged token-half-major
    # [IH, P, CB, 512]: each half is one contiguous 0.5MB block with 4KB
    # per-partition lines -> full-rate sequential DRAM reads, and the two
    # halves can ride different DGE rings concurrently.
    xts = nc.dram_tensor("xts", (B_LOC, IH, P, CB, 512), BF16, kind="ExternalInput").ap()
    x2ts = nc.dram_tensor("x2ts", (B_LOC, IH, P, CB, 512), BF16, kind="ExternalInput").ap()
    wqt = nc.dram_tensor("wqt", (P, CB, C), BF16, kind="ExternalInput").ap()
    wkt = nc.dram_tensor("wkt", (P, CB, C), BF16, kind="ExternalInput").ap()
    wvt = nc.dram_tensor("wvt", (P, CB, C), BF16, kind="ExternalInput").ap()
    wpt = nc.dram_tensor("wpt", (P, CB, C), BF16, kind="ExternalInput").ap()
    bp = nc.dram_tensor("bp", (C,), F32, kind="ExternalInput").ap()
    y = nc.dram_tensor("y", (B_LOC, N, C), F32, kind="ExternalOutput").ap()

    with tile.TileContext(nc) as tc:
        with (
            tc.tile_pool(name="consts", bufs=1) as consts,
            tc.tile_pool(name="big", bufs=2) as big,
            tc.tile_pool(name="ptp", bufs=9) as ptp,
            tc.tile_pool(name="ypool", bufs=3) as ypool,
            tc.tile_pool(name="rpool", bufs=4) as rpool,
            tc.tile_pool(name="mmout", bufs=2, space="PSUM") as mmout,
            tc.tile_pool(name="stp", bufs=2, space="PSUM") as stp,
            tc.tile_pool(name="avp", bufs=2, space="PSUM") as avp,
        ):
            # Pre-warm the ACT exp table (~2.7us ACT_TABLE_LOAD) before any
            # scores exist, so the first real exp doesn't pay it.
            warm = consts.tile([1, 16], F32, tag="warm", name="warm")
            nc.vector.memset(warm, 0.0)

            dummy = consts.tile([P, 640], BF16, tag="dummy", name="dummy")
            nc.vector.memset(dummy, 0.125)

            # Weight SBUF tiles.
            wsb = {
                name: consts.tile([P, CB, C], BF16, tag=f"w_{name}", name=f"w_{name}")
                for name in ("wq", "wk", "wv", "wp")
            }

            # Input tiles [P, IH, CB, 512] per batch, loaded as two
            # half-tensor DMAs each.
            state = {}

            def in_tile(b, which):
                st = state.setdefault(b, {})
                if which not in st:
                    st[which] = big.tile(
                        [P, IH, CB, 512], BF16, tag=which, name=f"{which}_b{b}"
                    )
                return st[which]

            def dma_half(b, which, h, eng, gate=None):
                """Load one token half.  `gate`: a produced 2-element
                region; a corner copy from it into the destination makes
                the DMA trigger wait - the SDMA engines round-robin across
                ALL in-flight transfers on a ring, so an ungated transfer
                steals bandwidth from the critical lead-in set."""
                src = x2ts if which == "x2T" else xts
                t = in_tile(b, which)
                if gate is not None:
                    nc.vector.tensor_copy(t[0:1, h, 0, 0:2], gate)
                eng.dma_start(out=t[:, h], in_=src[b, h])

            # --- DMA plan.  Triggers are the first user instructions on
            # each ring so descriptors hit the queues the moment the
            # preamble barrier clears.  Critical wave (ungated): the four
            # 0.5MB chunks the first q/k projections need, two per HWDGE
            # ring so all land ~14us.  Everything else is corner-gated
            # (sync ring only - a gated trigger on the scalar ring would
            # block the exp stream) or rides the slow gpsimd SWDGE.
            nc.sync.dma_start(out=wsb["wq"], in_=wqt)
            nc.scalar.dma_start(out=wsb["wk"], in_=wkt)
            dma_half(0, "x2T", 0, nc.sync)
            dma_half(0, "xT", 0, nc.scalar)
            nc.gpsimd.dma_start(out=wsb["wv"], in_=wvt)

            bias_bc = consts.tile([P, C], F32, tag="bias_bc", name="bias_bc")
            nc.gpsimd.dma_start(
                out=bias_bc,
                in_=bass.AP(tensor=bp.tensor, offset=bp.offset, ap=[[0, P], [1, C]]),
            )
            nc.gpsimd.dma_start(out=wsb["wp"], in_=wpt)

            # ACT exp-table warm (scalar engine, after its dma triggers).
            warm2 = consts.tile([1, 16], F32, tag="warm2", name="warm2")
            nc.scalar.activation(warm2, warm, EXP, scale=SCALE)

            # Persistent per-(batch, token-block-PAIR) v tiles in fp8:
            # [P, 2 (m-subtile), H, ones|d].  Two m-blocks share a tile so
            # one fp8 DoubleRow matmul contracts 256 keys at 2x rate.  The
            # ones blocks still provide softmax denominators for free.
            VT = {
                b: [
                    consts.tile(
                        [P, 2, H, 2 * D], FP8, tag=f"VT{b}_{mp}", name=f"VT{b}_{mp}"
                    )
                    for mp in range(NB // 2)
                ]
                for b in range(B_LOC)
            }

            def vt_memset(b, mp):
                nc.vector.memset(VT[b][mp][:, :, :, 0:D], 1.0)

            for mp in range(NB // 2):
                vt_memset(0, mp)

            # Dummy-matmul bridge: keeps the PE HAM activity window filled
            # from preamble end (~3.6us) to the first real projection
            # (~14us at the cold 1.2GHz clock), so the clock flips to full
            # rate right as attention begins.
            # Dummy bridge: ~8.4us at the cold clock until the HAM flip
            # takes effect (~12us), then ~220ns each at full rate - ends
            # ~16.5us, just as the first critical input chunks land.  Any
            # idle gap here re-throttles the PE to 1.2GHz right as the
            # first real projections start.  Split into two chains: the
            # first doubles as the time-gate for the second-half input
            # loads.
            dps = mmout.tile([P, 512], F32, tag="mm", name="dps")
            for i in range(26):
                nc.tensor.matmul(
                    dps, dummy[:, 0:P], dummy[:, P : P + 512],
                    start=(i == 0), stop=(i == 25),
                )
            dpsb = mmout.tile([P, 512], F32, tag="mm", name="dpsb")
            for i in range(4):
                nc.tensor.matmul(
                    dpsb, dummy[:, 0:P], dummy[:, P : P + 512],
                    start=(i == 0), stop=(i == 3),
                )

            def qk_one(b, wname, kb, ih, cp=None):
                """One q/k projection step: [P, 512] of transposed output.
                (GPSIMD cannot read PSUM, so the copy has to ride the DVE.)"""
                if cp is None:
                    cp = nc.vector.tensor_copy
                st = state.setdefault(b, {})
                kind = "qT" if wname == "wq" else "kT"
                skey = "x2T" if wname == "wq" else "xT"
                dst = st.setdefault(kind, {})
                if kb not in dst:
                    dst[kb] = big.tile(
                        [P, N], BF16, tag=f"{kind}{kb}", name=f"{kind}{kb}_b{b}"
                    )

                def qk_step():
                    srcT = state[b][skey]
                    ps = mmout.tile(
                        [P, 512], F32, tag="mm", name=f"ps_{kind}_{b}_{kb}_{ih}"
                    )
                    for cb in range(CB):
                        nc.tensor.matmul(
                            ps,
                            wsb[wname][:, cb, kb * P : (kb + 1) * P],
                            srcT[:, ih, cb, :],
                            start=(cb == 0),
                            stop=(cb == CB - 1),
                        )
                    cp(dst[kb][:, ih * 512 : (ih + 1) * 512], ps)

                return qk_step

            def qk_group(b, kb, cp=None):
                return [
                    qk_one(b, "wq", kb, 0, cp), qk_one(b, "wq", kb, 1, cp),
                    qk_one(b, "wk", kb, 0, cp), qk_one(b, "wk", kb, 1, cp),
                ]

            def v_steps(b, nbs):
                """v projection, natural [n, (h, ones|d)] into VT[b]."""
                steps = []
                for nb in nbs:

                    def v_step(nb=nb):
                        ps = mmout.tile([P, C], F32, tag="mm", name=f"ps_v_{b}_{nb}")
                        h, loc = nb // 4, nb % 4
                        for cb in range(CB):
                            nc.tensor.matmul(
                                ps,
                                state[b]["xT"][:, h, cb, loc * P : (loc + 1) * P],
                                wsb["wv"][:, cb, :],
                                start=(cb == 0),
                                stop=(cb == CB - 1),
                            )
                        nc.vector.tensor_copy(
                            VT[b][nb // 2][:, nb % 2, :, D : 2 * D],
                            ps.rearrange("p (h d) -> p h d", h=H),
                        )

                    steps.append(v_step)
                return steps

            # --- attention machinery -------------------------------------
            def get_aT(b, hp):
                st = state[b]
                aT = st.setdefault("aT", {})
                if hp not in aT:
                    aT[hp] = big.tile([P, N], BF16, tag=f"aT{hp}", name=f"aT{hp}_b{b}")
                return aT[hp]

            def make_iter(b, hp, ih):
                """Allocate the PSUM accumulators + closures for one
                (head-pair, query-half) iteration."""
                get_aT(b, hp)
                avA = avp.tile([P, 512], F32, tag="av", name=f"avA_{b}_{hp}_{ih}")
                avB = avp.tile([P, 512], F32, tag="av", name=f"avB_{b}_{hp}_{ih}")
                sts = {}
                pts = {}

                def st_step(m):
                    kTt = state[b]["kT"][hp]
                    qTt = state[b]["qT"][hp]
                    isl = slice(ih * 512, (ih + 1) * 512)
                    msl = slice(m * P, (m + 1) * P)
                    st2 = stp.tile([P, 1024], F32, tag="st", name=f"st_{b}_{hp}_{ih}_{m}")
                    sts[m] = st2
                    nc.tensor.matmul(
                        st2[:, 0:512], kTt[0:D, msl], qTt[0:D, isl],
                        start=True, stop=True,
                    )
                    nc.tensor.matmul(
                        st2[:, 512:1024], kTt[D : 2 * D, msl],
                        qTt[D : 2 * D, isl], start=True, stop=True,
                    )

                def exp_step(m):
                    # exp writes fp8e4 directly into one m-subtile of a
                    # [P, 2, 1024] pair tile (values in [~0.2, 5] - well
                    # inside e4m3 range).
                    mp = m // 2
                    if m % 2 == 0:
                        pts[mp] = ptp.tile(
                            [P, 2, 1024], FP8, tag="pt", name=f"pt_{b}_{hp}_{ih}_{mp}"
                        )
                    nc.scalar.activation(pts[mp][:, m % 2, :], sts.pop(m), EXP,
                                         scale=SCALE)

                def pv_step(mp):
                    # fp8 DoubleRow: one matmul contracts both m-subtiles
                    # (256 keys) at 2x rate - halves the PV slot count on
                    # the bottleneck PE.
                    pt2 = pts.pop(mp)
                    nc.tensor.matmul(
                        avA, VT[b][mp][:, :, 2 * hp, :], pt2[:, :, 0:512],
                        start=(mp == 0), stop=(mp == NB // 2 - 1), perf_mode=DR,
                    )
                    nc.tensor.matmul(
                        avB, VT[b][mp][:, :, 2 * hp + 1, :], pt2[:, :, 512:1024],
                        start=(mp == 0), stop=(mp == NB // 2 - 1), perf_mode=DR,
                    )

                rA = rpool.tile([D, 512], F32, tag="recip", name=f"rA_{b}_{hp}_{ih}")
                rB = rpool.tile([D, 512], F32, tag="recip", name=f"rB_{b}_{hp}_{ih}")

                def norm_step():
                    # approx reciprocal: ~18 correct bits, ~5x faster than
                    # the exact DVE reciprocal.  Denominators sit at PSUM
                    # partitions 0-63 (ones block is first in v tiles).
                    isl = slice(ih * 512, (ih + 1) * 512)
                    aTt = state[b]["aT"][hp]
                    nc.vector.reciprocal_approx_fast(out=rA, in_=avA[0:D, :])
                    nc.vector.tensor_mul(aTt[0:D, isl], avA[D : 2 * D, :], rA)
                    nc.vector.reciprocal_approx_fast(out=rB, in_=avB[0:D, :])
                    nc.vector.tensor_mul(aTt[D : 2 * D, isl], avB[D : 2 * D, :], rB)

                def norm_recips():
                    nc.vector.reciprocal_approx_fast(out=rA, in_=avA[0:D, :])
                    nc.vector.reciprocal_approx_fast(out=rB, in_=avB[0:D, :])

                def norm_mul_chunk(j):
                    # one 128-token chunk of the normalization - lets the
                    # final output projections start ~1.5us earlier and
                    # pipeline with the rest of the norm.
                    aTt = state[b]["aT"][hp]
                    csl = slice(j * P, (j + 1) * P)
                    asl = slice(ih * 512 + j * P, ih * 512 + (j + 1) * P)
                    nc.vector.tensor_mul(aTt[0:D, asl], avA[D : 2 * D, csl], rA[:, csl])
                    nc.vector.tensor_mul(aTt[D : 2 * D, asl], avB[D : 2 * D, csl], rB[:, csl])

                return st_step, exp_step, pv_step, norm_step, norm_recips, norm_mul_chunk

            def attention_steps(iter_list):
                """Unified emission for a sequence of iterations across
                batches.  iter_list: [(b, hp, ih, defer), ...].  9 main
                steps per iteration.  Two scheduling tricks keep the ACT
                exp stream gap-free:
                  - the NEXT iteration's ST(0) is hoisted into this
                    iteration's m==7 step, BEFORE PV(7) and any fills, so
                    exp(it+1, 0) can start the moment exp(it, 7) ends;
                  - PV(0..1) are emitted after ST(2), so their wait on the
                    previous iteration's norm (which frees the PSUM
                    accumulators) never delays an ST.
                A deferred iteration emits no PVs until after exp(7) - used
                for b0's first iteration whose v tiles (wv on the slow
                SWDGE ring) arrive mid-iteration."""
                its = [make_iter(b, hp, ih) for (b, hp, ih, _) in iter_list]
                steps = []
                nit = len(iter_list)
                for k, ((b, hp, ih, defer), it) in enumerate(zip(iter_list, its)):
                    st_s, exp_s, pv_s, norm_s, recips_s, mulc_s = it
                    if k == nit - 1:
                        last_handles["mulc"] = mulc_s
                        norm_s = recips_s
                    first = (k == 0)
                    nxt_st = its[k + 1][0] if k + 1 < len(its) else None
                    for m in range(NB):
                        def step(m=m, st_s=st_s, exp_s=exp_s, pv_s=pv_s,
                                 defer=defer, first=first, nxt_st=nxt_st):
                            if m > 0 or first:
                                st_s(m)
                            exp_s(m)
                            if m == NB - 1 and nxt_st is not None:
                                nxt_st(0)
                            # PV(mp) trails the second exp of its m-pair by
                            # two steps, so the in-order PE queue never
                            # waits on an exp completion (a wait = a
                            # pipeline-drain burst break, ~160ns each, and
                            # PE is the bottleneck engine).
                            if not defer and m >= 3 and m % 2 == 1:
                                pv_s((m - 3) // 2)
                        steps.append(step)

                    def tail(defer=defer, pv_s=pv_s, norm_s=norm_s):
                        if defer:
                            for mp in range(NB // 2 - 1):
                                pv_s(mp)
                        pv_s(NB // 2 - 1)
                        norm_s()
                    steps.append(tail)
                return steps

            def proj_steps(b, nbs):
                """One step per output tile: 4 matmuls + bias + store on the
                sync HWDGE ring (engine idle; the gpsimd SWDGE drain cost
                ~3us of tail in v2)."""
                steps = []
                for nb in nbs:

                    def p_step(nb=nb):
                        ps = mmout.tile([P, C], F32, tag="mm", name=f"ps_y_{b}_{nb}")
                        for cb in range(CB):
                            nc.tensor.matmul(
                                ps,
                                state[b]["aT"][cb][:, nb * P : (nb + 1) * P],
                                wsb["wp"][:, cb, :],
                                start=(cb == 0),
                                stop=(cb == CB - 1),
                            )
                        ytile = ypool.tile([P, C], F32, tag="yt", name=f"yt_{b}_{nb}")
                        nc.vector.tensor_add(ytile, ps, bias_bc)
                        nc.sync.dma_start(
                            out=y[b, nb * P : (nb + 1) * P, :], in_=ytile
                        )

                    steps.append(p_step)
                return steps

            def run_interleaved(main_steps, fill_specs):
                """Emit main_steps with fills (step, deadline, not_before)
                distributed evenly, subject to: fill j MUST be emitted
                before main[deadline] (producers have to precede their
                consumers in the per-engine emission order or the consumer
                reads the previous run's stale buffer contents - no
                dependency is created on a not-yet-emitted producer), and
                MUST NOT be emitted before main[not_before] (the reverse
                hazard: a fill that READS data must follow its producers).
                Deadlines must be non-decreasing in list order."""
                main = list(main_steps)
                fills = list(fill_specs)
                nf = len(fills)
                done = 0
                for i, s in enumerate(main):
                    while done < nf and fills[done][1] <= i:
                        fills[done][0]()
                        done += 1
                    s()
                    while done < nf and fills[done][2] <= i + 1:
                        fills[done][0]()
                        done += 1
                while done < nf:
                    fills[done][0]()
                    done += 1

            # --- emission schedule ---------------------------------------
            # The second token halves are gated on the first dummy chain's
            # PSUM output - a pure time gate that releases right as the
            # critical chunks finish (~16us), so they never round-robin
            # against them.  Both triggers ride the sync engine: a gated
            # trigger on the scalar engine would block the q/k copies and
            # the exp stream behind it.
            dma_half(0, "xT", 1, nc.sync, gate=dps[0:1, 0:2])
            dma_half(0, "x2T", 1, nc.sync, gate=dps[0:1, 2:4])

            # Serial prologue: q/k for head-pair 0, query/token half 0 only
            # (x2T.h0 + xT.h0 land ~14us; copies on the idle ACT engine).
            qk_one(0, "wq", 0, 0, cp=nc.scalar.copy)()
            qk_one(0, "wk", 0, 0, cp=nc.scalar.copy)()

            # All 16 iterations in one interleave: b0 hp-outer (iteration 1
            # deferred behind wv's slow arrival), then b1 ih0 x 4 hp, then
            # b1 ih1 x 4 hp.  Iteration k spans mains 9k..9k+8; the st0 of
            # iteration k+1 is emitted inside main 9k+7.
            iter_list = [(0, hp, ih, hp == 0 and ih == 0)
                         for hp in range(CB) for ih in range(IH)]
            iter_list += [(1, hp, 0, False) for hp in range(CB)]
            iter_list += [(1, hp, 1, False) for hp in range(CB)]
            last_handles = {}

            b1gate = VT[0][NB // 2 - 1][0:1, 1, 0, D : D + 2]
            vs0 = v_steps(0, range(NB))
            vs1 = v_steps(1, range(NB))
            pj0 = proj_steps(0, range(NB))
            pj1 = proj_steps(1, range(NB))

            # Fill specs (step, deadline, target): targets are explicit main
            # indices chosen so each phase's fill load matches its spare PE
            # capacity (~3.3 fill steps per iteration window); deadlines
            # are the emission-order correctness bounds.
            F = []
            F.append((qk_one(0, "wk", 0, 1), 4, 1))
            F.append((vs0[0], 6, 2))
            F.append((vs0[1], 6, 3))
            F.append((qk_one(0, "wq", 0, 1), 7, 4))
            for j, m in enumerate(range(2, NB)):
                F.append((vs0[m], 8, 4 + j // 2))
            for wh, h in (("x2T", 0), ("x2T", 1), ("xT", 0), ("xT", 1)):
                F.append((lambda wh=wh, h=h:
                          dma_half(1, wh, h, nc.sync, gate=b1gate), 16, 8))
            F += [(s, 16, 10 + j) for j, s in enumerate(qk_group(0, 1))]
            F += [(lambda mp=mp: vt_memset(1, mp), 33, 14 + mp) for mp in range(2)]
            F += [(s, 33, 19 + 2 * j) for j, s in enumerate(qk_group(0, 2))]
            F += [(lambda mp=mp: vt_memset(1, mp), 51, 22 + mp) for mp in range(2, 4)]
            F += [(s, 51, 31 + 2 * j) for j, s in enumerate(qk_group(0, 3))]
            F += [(s, 69, 39 + 2 * j) for j, s in enumerate(qk_group(1, 0))]
            # v(1, nb) must precede PV(nb//2) of b1's first iteration:
            # PV(mp) is emitted at main 72 + (2*mp + 3), PV(3) in the tail
            # step (main 80).
            F += [(vs1[m], 73, 47 + 2 * m) for m in range(4)]
            F += [(s, 78, [55, 58, 61, 64][j]) for j, s in enumerate(qk_group(1, 1))]
            F += [(vs1[m], [79, 79, 80, 80][m - 4], [66, 68, 73, 75][m - 4])
                  for m in range(4, NB)]
            F += [(s, 87, [78, 81, 84, 86][j]) for j, s in enumerate(qk_group(1, 2))]
            F += [(s, 96, 89 + 2 * j) for j, s in enumerate(qk_group(1, 3))]
            F += [(pj0[j], 144, 109 + 3 * j) for j in range(6)]
            F.append((pj1[0], 144, 126))
            F.append((pj0[6], 144, 127))
            F.append((pj1[1], 144, 130))
            F.append((pj0[7], 144, 130))
            F.append((pj1[2], 144, 134))
            F.append((pj1[3], 144, 138))
            run_interleaved(attention_steps(iter_list), F)

            # Keep the PE busy through the final recips' DVE window (an
            # idle PE can straddle a HAM MID window and re-throttle to
            # 1.2GHz, making the projection tail run cold), then drain the
            # last four output tiles chunk-by-chunk behind the norm muls.
            dps2 = mmout.tile([P, 512], F32, tag="mm", name="dps2")
            for i in range(8):
                nc.tensor.matmul(
                    dps2, dummy[:, 0:P], dummy[:, P : P + 512],
                    start=(i == 0), stop=(i == 7),
                )
            for j in range(4):
                last_handles["mulc"](j)
                pj1[4 + j]()

    nc.compile()
    return nc


def _get_nc():
    if "nc" not in _CACHE:
        _CACHE["nc"] = _build_program()
    return _CACHE["nc"]


def _get_runner():
    """Build (once) a jitted 8-core shard_map executor for the program."""
    if "runner" in _CACHE:
        return _CACHE["runner"]

    import jax
    from jax.experimental.shard_map import shard_map
    from jax.sharding import Mesh, PartitionSpec

    from concourse import bass2jax as b2j

    nc = _get_nc()
    b2j.install_neuronx_cc_hook()
    assert nc.dbg_addr is None
    partition_name = nc.partition_id_tensor.name if nc.partition_id_tensor else None

    in_names = []
    out_names = []
    out_avals = []
    zero_outs = []
    for alloc in nc.m.functions[0].allocations:
        if not isinstance(alloc, mybir.MemoryLocationSet):
            continue
        name = alloc.memorylocations[0].name
        if alloc.kind == "ExternalInput":
            if name != partition_name:
                in_names.append(name)
        elif alloc.kind == "ExternalOutput":
            out_names.append(name)
            shape = tuple(alloc.tensor_shape)
            dtype = mybir.dt.np(alloc.dtype)
            out_avals.append(jax.core.ShapedArray(shape, dtype))
            zero_outs.append(np.zeros(shape, dtype))
    n_params = len(in_names)
    all_names = in_names + out_names
    if partition_name is not None:
        all_names = all_names + [partition_name]

    def _body(*args):
        operands = list(args)
        if partition_name is not None:
            operands.append(b2j.partition_id_tensor())
        outs = b2j._bass_exec_p.bind(
            *operands,
            out_avals=tuple(out_avals),
            in_names=tuple(all_names),
            out_names=tuple(out_names),
            lowering_input_output_aliases=(),
            sim_require_finite=True,
            sim_require_nnan=True,
            nc=nc,
        )
        return tuple(outs)

    devices = jax.devices()[:NCORES]
    mesh = Mesh(np.asarray(devices), ("core",))
    n_outs = len(out_names)
    sharded = jax.jit(
        shard_map(
            _body,
            mesh=mesh,
            in_specs=(PartitionSpec("core"),) * (n_params + n_outs),
            out_specs=(PartitionSpec("core"),) * n_outs,
            check_rep=False,
        ),
        donate_argnums=tuple(range(n_params, n_params + n_outs)),
        keep_unused=True,
    )

    def run(in_maps):
        concat_in = [
            np.concatenate([np.asarray(m[name]) for m in in_maps], axis=0)
            for name in in_names
        ]
        concat_zeros = [
            np.zeros((NCORES * z.shape[0], *z.shape[1:]), z.dtype) for z in zero_outs
        ]
        out_arrs = sharded(*concat_in, *concat_zeros)
        return [
            {
                name: np.asarray(out_arrs[i]).reshape(NCORES, *out_avals[i].shape)[c]
                for i, name in enumerate(out_names)
            }
            for c in range(NCORES)
        ]

    _CACHE["runner_parts"] = dict(
        sharded=sharded,
        in_names=in_names,
        out_names=out_names,
        out_avals=out_avals,
        zero_outs=zero_outs,
        mesh=mesh,
    )
    _CACHE["runner"] = run
    return run


def make_in_maps(x, x2, Wq, Wk, Wv, Wp, bp):
    """Host-side prep shared by kernel() and test harnesses: shard the
    batch; pre-transpose x/x2 to [C, N] bf16, partition-major, split into
    two contiguous 512-token halves [IH, P, CB, 512]; weights pre-
    transposed and arranged [P, CB, C]."""
    import ml_dtypes

    bf16 = ml_dtypes.bfloat16

    def arrange_x(a):
        # [B, N, C] -> [B, C, N] -> [B, CB, P, IH, 512] -> [B, IH, P, CB, 512]
        a = np.asarray(a, dtype=np.float32).astype(bf16).transpose(0, 2, 1)
        a = a.reshape(a.shape[0], CB, P, IH, 512)
        return np.ascontiguousarray(a.transpose(0, 3, 2, 1, 4))

    def arrange_w(w):
        # W [C, C] -> W.T -> [CB, P, C] -> [P, CB, C]
        wt = np.asarray(w, dtype=np.float32).T.astype(bf16)
        return np.ascontiguousarray(wt.reshape(CB, P, C).transpose(1, 0, 2))

    xt = arrange_x(x)
    x2t = arrange_x(x2)
    wqt = arrange_w(Wq)
    wkt = arrange_w(Wk)
    wvt = arrange_w(Wv)
    wpt = arrange_w(Wp)
    bp = np.asarray(bp, dtype=np.float32)

    in_maps = []
    for c in range(NCORES):
        in_maps.append(
            {
                "xts": xt[c * B_LOC : (c + 1) * B_LOC],
                "x2ts": x2t[c * B_LOC : (c + 1) * B_LOC],
                "wqt": wqt,
                "wkt": wkt,
                "wvt": wvt,
                "wpt": wpt,
                "bp": bp,
            }
        )
    return in_maps


def kernel(x, x2, Wq, Wk, Wv, Wp, bp):
    in_maps = make_in_maps(x, x2, Wq, Wk, Wv, Wp, bp)
    if os.environ.get("KERNEL_RUNNER", "cached") == "spmd":
        res = run_bass_kernel_spmd(_get_nc(), in_maps, core_ids=list(range(NCORES)))
        results = res.results
    else:
        run = _get_runner()
        results = run(in_maps)
    out = np.concatenate([r["y"] for r in results], axis=0)
    return out.astype(np.float32)
